# revision 17
# baseline (speedup 1.0000x reference)
"""Trainium2 Bass kernel for nn_GCNII_80178449482260 (2x dense GAT + GCNII).

Row-parallel over N=1024 across 8 cores (128 rows each), restructured to
minimize collectives and keep the PE continuously busy (p-state ramp):

  * GAT layers use  att @ Wh == (att @ h_in) @ W  and
    u,v == h_in @ (W @ a_half)  with W@a precomputed on host.  Since x is
    replicated, the five GAT1 heads need NO big all-gather -- just one tiny
    [5,128] v-gather.  GAT2's two heads share one input all-gather.
  * Collectives (6): v1(tiny), h0, [Wh_o1|v], [xg|v], [hcat2|v], support.
    All outputs in Shared DRAM; v vectors ride inside the payloads.
  * All [128, k*128] transposes via DMA XBAR (dma_start_transpose) on the
    two HWDGE rings (sync + scalar) -- zero PE/DVE cost.
  * Scalar engine runs ONLY Exp; lrelu/elu masks on DVE (bf16 where safe);
    PSUM->SBUF copies on DVE (gpsimd has no PSUM port).
  * Weights stream as single big DMAs (split across 16 SDMA engines).
  * Per-head software pipelining: head h+1's softmax chain and the o1-Wh
    accumulation of head h-1 are interleaved into head h's matmul stream.
"""
import os
import sys
import numpy as np

for _p in ("/opt/trn_rl_repo", "/root/.axon_site/_ro/trn_rl_repo"):
    if _p not in sys.path:
        sys.path.insert(0, _p)

import ml_dtypes  # noqa: E402
from concourse import bacc, tile, mybir  # noqa: E402
from concourse.bass_utils import run_bass_kernel_spmd  # noqa: E402
from concourse.kernels.tile_matmul import make_identity  # noqa: E402

BF16 = mybir.dt.bfloat16
F32 = mybir.dt.float32
AF = mybir.ActivationFunctionType
OP = mybir.AluOpType

N = 1024      # nodes
P = 128       # partitions / rows per core
C = 8         # cores
HID = 512
NC1 = 512
H1, H2 = 5, 2
THETA2 = float(np.log(1.25))   # GCNII layer-2 theta; layer 1 is dead code
SLOPE = 0.25
GSLOPE = 0.01                  # GAT leaky-relu slope
RG = [list(range(C))]

_CACHE = {}


def _build(reps=1):
    nc = bacc.Bacc("TRN2", target_bir_lowering=False, debug=False,
                   num_devices=C)
    d = {}
    d["xT_sl"] = nc.dram_tensor("xT_sl", [N, P], BF16, kind="ExternalInput")
    d["x_row"] = nc.dram_tensor("x_row", [N, N], BF16, kind="ExternalInput")
    d["adj_r"] = nc.dram_tensor("adj_r", [P, N], F32, kind="ExternalInput")
    d["Wg1"] = nc.dram_tensor("Wg1", [H1, N, N], BF16, kind="ExternalInput")
    d["Wa1"] = nc.dram_tensor("Wa1", [N, 2 * H1], BF16, kind="ExternalInput")
    d["Wo1"] = nc.dram_tensor("Wo1", [H1 * N, NC1], BF16, kind="ExternalInput")
    d["ao1"] = nc.dram_tensor("ao1", [2 * NC1], BF16, kind="ExternalInput")
    d["Wg2"] = nc.dram_tensor("Wg2", [H2, NC1, NC1], BF16, kind="ExternalInput")
    d["Wa2"] = nc.dram_tensor("Wa2", [NC1, 2 * H2], BF16, kind="ExternalInput")
    d["Wo2"] = nc.dram_tensor("Wo2", [N, N], BF16, kind="ExternalInput")
    d["Woa2"] = nc.dram_tensor("Woa2", [N, 2], BF16, kind="ExternalInput")
    d["fc0_w"] = nc.dram_tensor("fc0_w", [N, HID], BF16, kind="ExternalInput")
    d["fc0_b"] = nc.dram_tensor("fc0_b", [HID], BF16, kind="ExternalInput")
    d["fc1_w"] = nc.dram_tensor("fc1_w", [HID, N], BF16, kind="ExternalInput")
    d["fc1_b"] = nc.dram_tensor("fc1_b", [N], BF16, kind="ExternalInput")
    d["cw1T_sl"] = nc.dram_tensor("cw1T_sl", [N, P], BF16, kind="ExternalInput")
    out_d = nc.dram_tensor("out", [P, N], F32, kind="ExternalOutput")
    dbg = {}
    if os.environ.get("KDEBUG"):
        for nm, shp, dt in [("d_h0f", [P, HID], F32), ("d_uv1", [P, 2 * H1], F32),
                            ("d_n0", [P, N], BF16),
                            ("d_z0", [P, N], BF16), ("d_o0", [P, N], BF16),
                            ("d_wh", [P, NC1], BF16), ("d_uvo1", [P, 2], F32),
                            ("d_xg", [P, NC1 + 8], BF16),
                            ("d_uv2", [P, 2 * H2], F32),
                            ("d_hc2", [P, N + 16], BF16),
                            ("d_n20", [P, N], BF16), ("d_z20", [P, NC1], BF16),
                            ("d_xg2", [P, N], BF16), ("d_sf", [P, HID], F32)]:
            dbg[nm] = nc.dram_tensor(nm, shp, dt, kind="ExternalOutput")

    with tile.TileContext(nc) as tc:
        _body(nc, tc, d, out_d, reps, dbg)
    nc.compile()
    return nc


def _body(nc, tc, d, out_d, reps=1, dbg=None):
    dbg = dbg or {}

    def dtap(nm, ap):
        if nm in dbg:
            nc.sync.dma_start(dbg[nm].ap(), ap)

    with (
        tc.tile_pool(name="cst", bufs=1) as cst,          # constants
        tc.tile_pool(name="per", bufs=1) as per,          # cross-phase persistents
        tc.tile_pool(name="wstr", bufs=2) as w_str,       # big weight stream
        tc.tile_pool(name="full", bufs=2) as full_p,      # gathered full tensors
        tc.tile_pool(name="att", bufs=2) as att_p,        # per-layer attention
        tc.tile_pool(name="scbf", bufs=2) as sc_bf,       # bf16 scratch
        tc.tile_pool(name="sc32", bufs=2) as sc_32,       # f32 scratch
        tc.tile_pool(name="sm", bufs=2) as sm,            # small vectors
        tc.tile_pool(name="psbig", bufs=2, space="PSUM") as ps_big,   # [128,1024] f32
        tc.tile_pool(name="pswh", bufs=1, space="PSUM") as ps_wh,     # [128,512] f32
        tc.tile_pool(name="pstr", bufs=2, space="PSUM") as ps_tr,     # [128,128] bf16
        tc.tile_pool(name="psuv", bufs=1, space="PSUM") as ps_uv,     # tiny f32
        tc.tile_pool(name="dram", bufs=1, space="DRAM") as dram,
    ):
        ident = cst.tile([P, P], BF16, tag="ident")
        make_identity(nc, ident)

        # ---------- constant / persistent loads ----------
        xT_sb = per.tile([P, C, P], BF16, tag="xT")
        nc.scalar.dma_start(xT_sb[:], d["xT_sl"].ap().rearrange("(c p) m -> p c m", p=P))
        x_row = per.tile([P, C, N], BF16, tag="x_row")
        nc.scalar.dma_start(x_row[:], d["x_row"].ap().rearrange("(c p) f -> p c f", p=P))
        wa1_sb = cst.tile([P, C, 2 * H1], BF16, tag="wa1")
        nc.scalar.dma_start(wa1_sb[:], d["Wa1"].ap().rearrange("(c p) f -> p c f", p=P))
        fc0_sb = cst.tile([P, C, HID], BF16, tag="fc0")
        nc.scalar.dma_start(fc0_sb[:], d["fc0_w"].ap().rearrange("(c p) f -> p c f", p=P))
        wa2_sb = cst.tile([P, 4, 2 * H2], BF16, tag="wa2")
        nc.scalar.dma_start(wa2_sb[:], d["Wa2"].ap().rearrange("(c p) f -> p c f", p=P))
        cw1T_sb = cst.tile([P, C, P], BF16, tag="cw1T")
        nc.scalar.dma_start(cw1T_sb[:], d["cw1T_sl"].ap().rearrange("(c p) m -> p c m", p=P))

        adj_sb = sc_32.tile([P, N], F32, tag="s32")
        nc.scalar.dma_start(adj_sb[:], d["adj_r"].ap())
        madj = per.tile([P, N], BF16, tag="madj")        # 0 where adj>0 else -9e15
        nc.vector.tensor_scalar(madj[:], adj_sb[:], 0.0, None, op0=OP.is_gt)
        nc.vector.tensor_scalar(madj[:], madj[:], 1.0, 9e15,
                                op0=OP.subtract, op1=OP.mult)

        def bcast_const(L, src_ap, tag):
            row = sm.tile([1, N], BF16, tag="vrow", bufs=1)
            nc.gpsimd.dma_start(row[:1, :L], src_ap[None, :])
            bc = cst.tile([P, L], BF16, tag=f"bc_{tag}")
            nc.gpsimd.partition_broadcast(bc[:], row[:1, :L])
            return bc

        ao1_bc = bcast_const(2 * NC1, d["ao1"].ap(), "ao1")
        woa2u_bc = bcast_const(N, d["Woa2"].ap()[:, 0], "w2u")
        woa2v_bc = bcast_const(N, d["Woa2"].ap()[:, 1], "w2v")
        fc0b_bc = bcast_const(HID, d["fc0_b"].ap(), "f0b")
        fc1b_bc = bcast_const(N, d["fc1_b"].ap(), "f1b")

        # ---------------- helpers ----------------
        def allgather(src_sb, rows, cols, tag):
            ag_in = dram.tile([rows, cols], BF16, tag=f"agi_{tag}")
            ag_out = dram.tile([C * rows, cols], BF16, tag=f"ago_{tag}",
                               addr_space="Shared")
            nc.gpsimd.dma_start(ag_in[:], src_sb)
            nc.gpsimd.collective_compute(
                "AllGather", OP.bypass, replica_groups=RG,
                ins=[ag_in.opt()], outs=[ag_out.opt()])
            return ag_out

        def dma_T(src_bf_2d, dst_3d, eng):
            """[128, k*128] -> [128, k, 128] chunked transpose via DMA XBAR."""
            eng.dma_start_transpose(dst_3d, src_bf_2d)

        def vb_broadcast(row_src_3d):
            """[1, C, 128] DRAM view -> [1,N] sbuf -> [128,N]."""
            vrow = sm.tile([1, N], BF16, tag="vrow", bufs=1)
            nc.sync.dma_start(vrow[:1].rearrange("o (c p) -> o c p", p=P),
                              row_src_3d)
            vb = att_p.tile([P, N], BF16, tag="vb", bufs=2)
            nc.gpsimd.partition_broadcast(vb[:], vrow[:1, :])
            return vb

        def col_extract_vb(full_3d_col, tag):
            """[128, C] column view of a gathered payload -> vb [128, N].

            transpose (PE, tiny) -> sbuf -> DRAM bounce -> broadcast."""
            tp = ps_tr.tile([P, P], BF16, tag="tr")
            nc.tensor.transpose(tp[:C, :P], full_3d_col, ident[:])
            v_sb = sm.tile([C, P], BF16, tag="vx", bufs=2)
            nc.vector.tensor_copy(v_sb[:], tp[:C, :P])
            v_dr = dram.tile([C, P], BF16, tag=f"vxd_{tag}")
            nc.sync.dma_start(v_dr[:], v_sb[:])
            return vb_broadcast(v_dr[:][None])

        def softmax_rows(u_ap, vb_ap, tagid):
            """n_bf, rs = exp(lrelu(u + v^T) masked), 1/rowsum."""
            e_bf = sc_bf.tile([P, N], BF16, tag="ebf")
            nc.vector.scalar_tensor_tensor(e_bf[:], vb_ap, u_ap, madj[:],
                                           op0=OP.add, op1=OP.add)
            nc.vector.scalar_tensor_tensor(e_bf[:], e_bf[:], GSLOPE, e_bf[:],
                                           op0=OP.mult, op1=OP.max)
            n_bf = att_p.tile([P, N], BF16, tag="nbf")
            ssum = sm.tile([P, 1], F32, tag=f"ss_{tagid}")
            nc.scalar.activation(n_bf[:], e_bf[:], AF.Exp, accum_out=ssum[:])
            rs = sm.tile([P, 1], F32, tag=f"rs_{tagid}")
            nc.vector.reciprocal(rs[:], ssum[:])
            return n_bf, rs

        def elu_store(o_ps, dst_bf, L, rs=None):
            """dst = elu(rs * o_ps); rs=None means already scaled."""
            m32 = sc_32.tile([P, N], F32, tag="s32")
            r32 = sc_32.tile([P, N], F32, tag="s32c", bufs=1)
            if rs is not None:
                nc.vector.tensor_scalar(m32[:, :L], o_ps, rs[:], 0.0,
                                        op0=OP.mult, op1=OP.min)
                nc.vector.tensor_scalar(r32[:, :L], o_ps, rs[:], 0.0,
                                        op0=OP.mult, op1=OP.max)
            else:
                nc.vector.tensor_scalar(m32[:, :L], o_ps, 0.0, None, op0=OP.min)
                nc.vector.tensor_scalar(r32[:, :L], o_ps, 0.0, None, op0=OP.max)
            g32 = sc_32.tile([P, N], F32, tag="s32b", bufs=1)
            nc.scalar.activation(g32[:, :L], m32[:, :L], AF.Exp)
            nc.vector.scalar_tensor_tensor(dst_bf, g32[:, :L], -1.0, r32[:, :L],
                                           op0=OP.add, op1=OP.add)

        # persistent per-rep tensors
        hcatT = per.tile([P, H1 * C, P], BF16, tag="hcatT")   # [128, 40, 128]
        h0f = per.tile([P, HID], F32, tag="h0f")
        uv1_sb = per.tile([P, 2 * H1], F32, tag="uv1")
        h0_full = per.tile([P, C, HID], BF16, tag="h0full")

        wg_view = [d["Wg1"].ap()[h].rearrange("(c p) f -> p c f", p=P)
                   for h in range(H1)]
        wo1_view = d["Wo1"].ap().rearrange("(g c p) f -> p g c f", p=P, c=C)
        wg2_view = [d["Wg2"].ap()[h].rearrange("(c p) f -> p c f", p=P)
                    for h in range(H2)]
        wo2_view = d["Wo2"].ap().rearrange("(c p) f -> p c f", p=P)
        fc1_view = d["fc1_w"].ap().rearrange("(c p) f -> p c f", p=P)

        for _rep in range(reps):
            # ======== GCNII h0 = lrelu(x@fc0 + b) ========
            h0_ps = ps_wh.tile([P, HID], F32, tag="wh")
            for c in range(C):
                nc.tensor.matmul(h0_ps[:], xT_sb[:, c, :], fc0_sb[:, c, :],
                                 start=(c == 0), stop=(c == C - 1))
            nc.vector.scalar_tensor_tensor(h0f[:], h0_ps[:], 1.0, fc0b_bc[:],
                                           op0=OP.mult, op1=OP.add)
            nc.vector.scalar_tensor_tensor(h0f[:], h0f[:], SLOPE, h0f[:],
                                           op0=OP.mult, op1=OP.max)
            h0b = sc_bf.tile([P, HID], BF16, tag="h0b")
            nc.vector.tensor_copy(h0b[:], h0f[:])
            dtap("d_h0f", h0f[:])

            # ======== GAT1 u,v for all heads: uv = x @ Wa1 ========
            uv1_ps = ps_uv.tile([P, 2 * H1], F32, tag="uv")
            for c in range(C):
                nc.tensor.matmul(uv1_ps[:], xT_sb[:, c, :], wa1_sb[:, c, :],
                                 start=(c == 0), stop=(c == C - 1))
            nc.vector.tensor_copy(uv1_sb[:], uv1_ps[:])
            dtap("d_uv1", uv1_sb[:])
            # v rows (cols H1..2H1) -> [5,128] for the tiny AG
            v1_bf = sc_bf.tile([P, 2 * H1], BF16, tag="v1bf")
            nc.vector.tensor_copy(v1_bf[:], uv1_sb[:])
            vtr_ps = ps_tr.tile([P, P], BF16, tag="tr")
            nc.tensor.transpose(vtr_ps[:2 * H1, :P], v1_bf[:], ident[:])
            vtr_sb = sm.tile([2 * H1, P], BF16, tag="vtr", bufs=1)
            nc.vector.tensor_copy(vtr_sb[:], vtr_ps[:2 * H1, :P])
            ag_v1 = allgather(vtr_sb[H1:2 * H1, :], H1, P, "v1")
            v1_rows = ag_v1[:].rearrange("(c h) p -> h c p", h=H1)

            # h0 allgather (result needed only in GCNII tail)
            ag_h0 = allgather(h0b[:], P, HID, "h0")
            nc.sync.dma_start(h0_full[:], ag_h0[:].rearrange("(c p) f -> p c f", p=P))

            # GAT1 weight stream: heads 0,1 prefetch now
            wg_sb = []
            for h in range(2):
                t = w_str.tile([P, C, N], BF16, tag="wstream")
                nc.sync.dma_start(t[:], wg_view[h])
                wg_sb.append(t)
            wo1_sb = []
            t = w_str.tile([P, C, NC1], BF16, tag="wo1stream")
            nc.sync.dma_start(t[:], wo1_view[:, 0])
            wo1_sb.append(t)

            wh_ps = ps_wh.tile([P, NC1], F32, tag="wh")   # o1 Wh accumulator

            # ======== GAT1: 5 heads, software-pipelined ========
            vbs = {0: vb_broadcast(v1_rows[0][None]),
                   1: vb_broadcast(v1_rows[1][None])}
            sm_state = {0: softmax_rows(uv1_sb[:, 0:1], vbs.pop(0)[:], "g1")}  # noqa
            attTs = {0: att_p.tile([P, C, P], BF16, tag="attT", name="attT0")}
            dma_T(sm_state[0][0][:], attTs[0][:], nc.sync)
            dtap("d_n0", sm_state[0][0][:])
            for h in range(H1):
                n_bf, rs = sm_state.pop(h)
                attT = attTs.pop(h)
                # z = att @ x_full
                z_ps = ps_big.tile([P, N], F32, tag="big")
                for j in range(C):
                    for s in range(2):
                        nc.tensor.matmul(z_ps[:, s * 512:(s + 1) * 512],
                                         attT[:, j, :], x_row[:, j, s * 512:(s + 1) * 512],
                                         start=(j == 0), stop=(j == C - 1))
                z_bf = sc_bf.tile([P, N], BF16, tag="zbf")
                nc.vector.tensor_scalar(z_bf[:], z_ps[:], rs[:], None, op0=OP.mult)
                if h == 0:
                    dtap("d_z0", z_bf[:])
                zT = att_p.tile([P, C, P], BF16, tag="zT")
                dma_T(z_bf[:], zT[:], nc.scalar)
                # o1-Wh accumulation chunks of the PREVIOUS head (fills the
                # z->o latency window on the PE)
                if h > 0:
                    for j in range(C):
                        nc.tensor.matmul(wh_ps[:], hcatT[:, (h - 1) * C + j, :],
                                         wo1_sb[h - 1][:, j, :],
                                         start=(h == 1 and j == 0), stop=False,
                                         skip_group_check=True)
                # next head's softmax + attT transpose (overlaps PE work)
                if h + 2 < H1:
                    vbs[h + 2] = vb_broadcast(v1_rows[h + 2][None])
                if h + 1 < H1:
                    sm_state[h + 1] = softmax_rows(uv1_sb[:, h + 1:h + 2],
                                                   vbs.pop(h + 1)[:], "g1")
                    attTs[h + 1] = att_p.tile([P, C, P], BF16, tag="attT",
                                              name=f"attT{h + 1}")
                    dma_T(sm_state[h + 1][0][:], attTs[h + 1][:], nc.sync)
                # out = z @ Wg1[h]
                o_ps = ps_big.tile([P, N], F32, tag="big")
                wgh = wg_sb[h]
                for j in range(C):
                    for s in range(2):
                        nc.tensor.matmul(o_ps[:, s * 512:(s + 1) * 512],
                                         zT[:, j, :], wgh[:, j, s * 512:(s + 1) * 512],
                                         start=(j == 0), stop=(j == C - 1))
                # weight prefetches
                if h + 2 < H1:
                    t = w_str.tile([P, C, N], BF16, tag="wstream")
                    nc.sync.dma_start(t[:], wg_view[h + 2])
                    wg_sb.append(t)
                if h + 1 < H1:
                    t = w_str.tile([P, C, NC1], BF16, tag="wo1stream")
                    nc.sync.dma_start(t[:], wo1_view[:, h + 1])
                    wo1_sb.append(t)
                o_bf = sc_bf.tile([P, N], BF16, tag="obf")
                elu_store(o_ps[:], o_bf[:], N)
                if h == 0:
                    dtap("d_o0", o_bf[:])
                dma_T(o_bf[:], hcatT[:, h * C:(h + 1) * C, :], nc.sync)
            # last head's o1-Wh chunks
            for j in range(C):
                nc.tensor.matmul(wh_ps[:], hcatT[:, (H1 - 1) * C + j, :],
                                 wo1_sb[H1 - 1][:, j, :],
                                 start=False, stop=(j == C - 1),
                                 skip_group_check=True)

            # ======== GAT1 out-attention (o1) ========
            junk = sc_bf.tile([P, N], BF16, tag="zbf")
            uvo1 = sm.tile([P, 2], F32, tag="uvo1")
            nc.vector.scalar_tensor_tensor(junk[:, :NC1], wh_ps[:], 1.0,
                                           ao1_bc[:, :NC1], op0=OP.mult,
                                           op1=OP.mult, accum_out=uvo1[:, 0:1])
            nc.vector.scalar_tensor_tensor(junk[:, :NC1], wh_ps[:], 1.0,
                                           ao1_bc[:, NC1:], op0=OP.mult,
                                           op1=OP.mult, accum_out=uvo1[:, 1:2])
            dtap("d_uvo1", uvo1[:])
            # payload [Wh | v | pad]
            pay_wh = sc_bf.tile([P, NC1 + 8], BF16, tag="pay520")
            nc.vector.tensor_copy(pay_wh[:, :NC1], wh_ps[:])
            nc.vector.tensor_copy(pay_wh[:, NC1:NC1 + 1], uvo1[:, 1:2])
            nc.vector.memset(pay_wh[:, NC1 + 1:], 0.0)
            dtap("d_wh", pay_wh[:, :NC1])
            ag_wh = allgather(pay_wh[:], P, NC1 + 8, "wh")
            wh_full = full_p.tile([P, C, NC1 + 8], BF16, tag="full520")
            nc.sync.dma_start(wh_full[:], ag_wh[:].rearrange("(c p) f -> p c f", p=P))
            vb = col_extract_vb(wh_full[:, :, NC1], "o1")
            n_bf, rs = softmax_rows(uvo1[:, 0:1], vb[:], "o1")
            attT = att_p.tile([P, C, P], BF16, tag="attT")
            dma_T(n_bf[:], attT[:], nc.sync)
            xg_ps = ps_wh.tile([P, NC1], F32, tag="wh")
            for j in range(C):
                nc.tensor.matmul(xg_ps[:], attT[:, j, :], wh_full[:, j, :NC1],
                                 start=(j == 0), stop=(j == C - 1))
            # xg = elu(rs * xg_ps) -> payload [xg | v1 v2 | pad]
            pay_g2 = sc_bf.tile([P, NC1 + 8], BF16, tag="pay520")
            elu_store(xg_ps[:], pay_g2[:, :NC1], NC1, rs=rs)
            xgT = att_p.tile([P, 4, P], BF16, tag="xgT")
            dma_T(pay_g2[:, :NC1], xgT[:], nc.scalar)
            uv2_ps = ps_uv.tile([P, 2 * H2], F32, tag="uv")
            for c in range(4):
                nc.tensor.matmul(uv2_ps[:], xgT[:, c, :], wa2_sb[:, c, :],
                                 start=(c == 0), stop=(c == 3))
            uv2_sb = sm.tile([P, 2 * H2], F32, tag="uv2")
            nc.vector.tensor_copy(uv2_sb[:], uv2_ps[:])
            nc.vector.tensor_copy(pay_g2[:, NC1:NC1 + 2], uv2_sb[:, H2:])
            nc.vector.memset(pay_g2[:, NC1 + 2:], 0.0)
            dtap("d_xg", pay_g2[:])
            dtap("d_uv2", uv2_sb[:])
            ag_xg = allgather(pay_g2[:], P, NC1 + 8, "xg")
            xg_full = full_p.tile([P, C, NC1 + 8], BF16, tag="full520")
            nc.sync.dma_start(xg_full[:], ag_xg[:].rearrange("(c p) f -> p c f", p=P))
            # wg2 stream (needed from here on)
            wg2_sb = w_str.tile([P, H2, 4, NC1], BF16, tag="wo1stream")
            for h in range(H2):
                nc.sync.dma_start(wg2_sb[:, h], wg2_view[h])

            # ======== GAT2: 2 heads (pipelined) ========
            pay_o2 = sc_bf.tile([P, N + 16], BF16, tag="payo2", bufs=1)
            vbs2 = {h: col_extract_vb(xg_full[:, :, NC1 + h], f"g2_{h}")
                    for h in range(H2)}
            sm2 = {0: softmax_rows(uv2_sb[:, 0:1], vbs2.pop(0)[:], "g2")}
            attT2 = {0: att_p.tile([P, C, P], BF16, tag="attT", name="attT20")}
            dma_T(sm2[0][0][:], attT2[0][:], nc.sync)
            for h in range(H2):
                n_bf, rs = sm2.pop(h)
                if h == 0:
                    dtap("d_n20", n_bf[:])
                attT = attT2.pop(h)
                z_ps = ps_wh.tile([P, NC1], F32, tag="wh")
                for j in range(C):
                    nc.tensor.matmul(z_ps[:], attT[:, j, :],
                                     xg_full[:, j, :NC1],
                                     start=(j == 0), stop=(j == C - 1))
                z_bf = sc_bf.tile([P, NC1], BF16, tag="h0b")
                nc.vector.tensor_scalar(z_bf[:], z_ps[:], rs[:], None, op0=OP.mult)
                if h == 0:
                    dtap("d_z20", z_bf[:])
                zT = att_p.tile([P, 4, P], BF16, tag="xgT")
                dma_T(z_bf[:], zT[:], nc.scalar)
                if h + 1 < H2:
                    sm2[h + 1] = softmax_rows(uv2_sb[:, h + 1:h + 2],
                                              vbs2.pop(h + 1)[:], "g2")
                    attT2[h + 1] = att_p.tile([P, C, P], BF16, tag="attT",
                                               name=f"attT2{h + 1}")
                    dma_T(sm2[h + 1][0][:], attT2[h + 1][:], nc.sync)
                o_ps = ps_wh.tile([P, NC1], F32, tag="wh")
                for j in range(4):
                    nc.tensor.matmul(o_ps[:], zT[:, j, :], wg2_sb[:, h, j, :],
                                     start=(j == 0), stop=(j == 3))
                elu_store(o_ps[:], pay_o2[:, h * NC1:(h + 1) * NC1], NC1)

            # ======== GAT2 out-attention (o2) ========
            junk2 = sc_bf.tile([P, N], BF16, tag="zbf")
            uvo2 = sm.tile([P, 2], F32, tag="uvo2")
            nc.vector.scalar_tensor_tensor(junk2[:], pay_o2[:, :N], 1.0,
                                           woa2u_bc[:], op0=OP.mult,
                                           op1=OP.mult, accum_out=uvo2[:, 0:1])
            nc.vector.scalar_tensor_tensor(junk2[:], pay_o2[:, :N], 1.0,
                                           woa2v_bc[:], op0=OP.mult,
                                           op1=OP.mult, accum_out=uvo2[:, 1:2])
            nc.vector.tensor_copy(pay_o2[:, N:N + 1], uvo2[:, 1:2])
            nc.vector.memset(pay_o2[:, N + 1:], 0.0)
            dtap("d_hc2", pay_o2[:])
            ag_h2 = allgather(pay_o2[:], P, N + 16, "h2")
            h2_full = full_p.tile([P, C, N + 16], BF16, tag="h2full", bufs=1)
            nc.sync.dma_start(h2_full[:], ag_h2[:].rearrange("(c p) f -> p c f", p=P))
            # Wo2 stream (during the AG)
            wo2_sb = w_str.tile([P, C, N], BF16, tag="wstream")
            nc.sync.dma_start(wo2_sb[:], wo2_view)
            vb = col_extract_vb(h2_full[:, :, N], "o2")
            n_bf, rs = softmax_rows(uvo2[:, 0:1], vb[:], "o2")
            attT = att_p.tile([P, C, P], BF16, tag="attT")
            dma_T(n_bf[:], attT[:], nc.sync)
            # z = att @ hcat2_full
            z_ps = ps_big.tile([P, N], F32, tag="big")
            for j in range(C):
                for s in range(2):
                    nc.tensor.matmul(z_ps[:, s * 512:(s + 1) * 512],
                                     attT[:, j, :],
                                     h2_full[:, j, s * 512:(s + 1) * 512],
                                     start=(j == 0), stop=(j == C - 1))
            z_bf = sc_bf.tile([P, N], BF16, tag="zbf")
            nc.vector.tensor_scalar(z_bf[:], z_ps[:], rs[:], None, op0=OP.mult)
            zT = att_p.tile([P, C, P], BF16, tag="zT")
            dma_T(z_bf[:], zT[:], nc.scalar)
            o_ps = ps_big.tile([P, N], F32, tag="big")
            for j in range(C):
                for s in range(2):
                    nc.tensor.matmul(o_ps[:, s * 512:(s + 1) * 512],
                                     zT[:, j, :], wo2_sb[:, j, s * 512:(s + 1) * 512],
                                     start=(j == 0), stop=(j == C - 1))
            xg2_bf = sc_bf.tile([P, N], BF16, tag="obf")
            elu_store(o_ps[:], xg2_bf[:], N)
            dtap("d_xg2", xg2_bf[:])
            xg2T = att_p.tile([P, C, P], BF16, tag="zT")
            dma_T(xg2_bf[:], xg2T[:], nc.sync)

            # ======== GCNII ========
            hi_ps = ps_wh.tile([P, HID], F32, tag="wh")
            for j in range(C):
                nc.tensor.matmul(hi_ps[:], xg2T[:, j, :], h0_full[:, j, :],
                                 start=(j == 0), stop=(j == C - 1))
            sf = sc_32.tile([P, HID], F32, tag="sf", bufs=1)
            nc.vector.scalar_tensor_tensor(sf[:], hi_ps[:], 9.0, h0f[:],
                                           op0=OP.mult, op1=OP.add)
            nc.vector.tensor_scalar(sf[:], sf[:], 0.1, None, op0=OP.mult)
            s_bf = sc_bf.tile([P, HID], BF16, tag="h0b")
            nc.vector.tensor_copy(s_bf[:], sf[:])
            dtap("d_sf", sf[:])
            ag_s = allgather(s_bf[:], P, HID, "s")
            s_full = full_p.tile([P, C, HID], BF16, tag="sfull", bufs=1)
            nc.sync.dma_start(s_full[:], ag_s[:].rearrange("(c p) f -> p c f", p=P))
            fc1_sb = w_str.tile([P, 4, N], BF16, tag="wo1stream")
            nc.sync.dma_start(fc1_sb[:], fc1_view)
            mm_ps = ps_wh.tile([P, HID], F32, tag="wh")
            for c in range(C):
                nc.tensor.matmul(mm_ps[:], cw1T_sb[:, c, :], s_full[:, c, :],
                                 start=(c == 0), stop=(c == C - 1))
            hf = sc_32.tile([P, HID], F32, tag="s32")
            nc.vector.scalar_tensor_tensor(hf[:], sf[:], (1.0 - THETA2) / THETA2,
                                           mm_ps[:], op0=OP.mult, op1=OP.add)
            nc.vector.scalar_tensor_tensor(hf[:], hf[:], THETA2, h0f[:],
                                           op0=OP.mult, op1=OP.add)
            nc.vector.scalar_tensor_tensor(hf[:], hf[:], SLOPE, hf[:],
                                           op0=OP.mult, op1=OP.max)
            hb = sc_bf.tile([P, HID], BF16, tag="h0b")
            nc.vector.tensor_copy(hb[:], hf[:])
            hT = att_p.tile([P, 4, P], BF16, tag="xgT")
            dma_T(hb[:], hT[:], nc.scalar)
            y_ps = ps_big.tile([P, N], F32, tag="big")
            for c in range(4):
                for s in range(2):
                    nc.tensor.matmul(y_ps[:, s * 512:(s + 1) * 512], hT[:, c, :],
                                     fc1_sb[:, c, s * 512:(s + 1) * 512],
                                     start=(c == 0), stop=(c == 3))
            y_sb = sc_32.tile([P, N], F32, tag="s32")
            nc.vector.scalar_tensor_tensor(y_sb[:], y_ps[:], 1.0, fc1b_bc[:],
                                           op0=OP.mult, op1=OP.add)
            nc.sync.dma_start(out_d.ap(), y_sb[:])


def _shard_inputs(inputs):
    f32 = lambda a: np.asarray(a, dtype=np.float32)
    bf = lambda a: np.ascontiguousarray(f32(a)).astype(ml_dtypes.bfloat16)
    x = f32(inputs["x"])
    adj = f32(inputs["adj"])
    x_bf = bf(x)
    xT_bf = np.ascontiguousarray(x_bf.T)
    cw1T = np.ascontiguousarray(bf(inputs["cw1"]).T)
    Wg1 = f32(inputs["Wg1"])
    ag1 = f32(inputs["ag1"])[:, :, 0]          # [5, 2048]
    Wa1 = np.stack([Wg1[h] @ ag1[h, :N] for h in range(H1)] +
                   [Wg1[h] @ ag1[h, N:] for h in range(H1)], axis=1)  # [1024, 10]
    Wg2 = f32(inputs["Wg2"])
    ag2 = f32(inputs["ag2"])[:, :, 0]          # [2, 1024]
    Wa2 = np.stack([Wg2[h] @ ag2[h, :NC1] for h in range(H2)] +
                   [Wg2[h] @ ag2[h, NC1:] for h in range(H2)], axis=1)  # [512, 4]
    Wo2 = f32(inputs["Wo2"])
    ao2 = f32(inputs["ao2"])[:, 0]             # [2048]
    Woa2 = np.stack([Wo2 @ ao2[:N], Wo2 @ ao2[N:]], axis=1)  # [1024, 2]
    shared = {
        "x_row": x_bf,
        "Wg1": bf(Wg1),
        "Wa1": bf(Wa1),
        "Wo1": bf(inputs["Wo1"]),
        "ao1": bf(f32(inputs["ao1"])[:, 0]),
        "Wg2": bf(Wg2),
        "Wa2": bf(Wa2),
        "Wo2": bf(Wo2),
        "Woa2": bf(Woa2),
        "fc0_w": bf(inputs["fc0_w"]),
        "fc0_b": bf(inputs["fc0_b"]),
        "fc1_w": bf(inputs["fc1_w"]),
        "fc1_b": bf(inputs["fc1_b"]),
    }
    in_maps = []
    for c in range(C):
        r0, r1 = c * P, (c + 1) * P
        m = dict(shared)
        m["xT_sl"] = np.ascontiguousarray(xT_bf[:, r0:r1])
        m["adj_r"] = np.ascontiguousarray(adj[r0:r1])
        m["cw1T_sl"] = np.ascontiguousarray(cw1T[:, r0:r1])
        in_maps.append(m)
    return in_maps


def kernel(**inputs) -> np.ndarray:
    if "nc" not in _CACHE:
        _CACHE["nc"] = _build()
    nc = _CACHE["nc"]
    in_maps = _shard_inputs(inputs)
    res = run_bass_kernel_spmd(nc, in_maps, core_ids=list(range(C)))
    out = np.concatenate([res.results[c]["out"] for c in range(C)], axis=0)
    return np.asarray(out, dtype=np.float32)


# revision 18
# speedup vs baseline: 1.0943x; 1.0943x over previous
"""Trainium2 Bass kernel for nn_GCNII_80178449482260 (2x dense GAT + GCNII).

Row-parallel over N=1024 across 8 cores (128 rows each), restructured to
minimize collectives and keep the PE continuously busy (p-state ramp):

  * GAT layers use  att @ Wh == (att @ h_in) @ W  and
    u,v == h_in @ (W @ a_half)  with W@a precomputed on host.  Since x is
    replicated, the five GAT1 heads need NO big all-gather -- just one tiny
    [5,128] v-gather.  GAT2's two heads share one input all-gather.
  * Collectives (6): v1(tiny), h0, [Wh_o1|v], [xg|v], [hcat2|v], support.
    All outputs in Shared DRAM; v vectors ride inside the payloads.
  * All [128, k*128] transposes via DMA XBAR (dma_start_transpose) on the
    two HWDGE rings (sync + scalar) -- zero PE/DVE cost.
  * Scalar engine runs ONLY Exp; lrelu/elu masks on DVE (bf16 where safe);
    PSUM->SBUF copies on DVE (gpsimd has no PSUM port).
  * Weights stream as single big DMAs (split across 16 SDMA engines).
  * Per-head software pipelining: head h+1's softmax chain and the o1-Wh
    accumulation of head h-1 are interleaved into head h's matmul stream.
"""
import os
import sys
import numpy as np

for _p in ("/opt/trn_rl_repo", "/root/.axon_site/_ro/trn_rl_repo"):
    if _p not in sys.path:
        sys.path.insert(0, _p)

import ml_dtypes  # noqa: E402
from concourse import bacc, tile, mybir  # noqa: E402
from concourse.bass_utils import run_bass_kernel_spmd  # noqa: E402
from concourse.kernels.tile_matmul import make_identity  # noqa: E402

BF16 = mybir.dt.bfloat16
F32 = mybir.dt.float32
AF = mybir.ActivationFunctionType
OP = mybir.AluOpType

N = 1024      # nodes
P = 128       # partitions / rows per core
C = 8         # cores
HID = 512
NC1 = 512
H1, H2 = 5, 2
THETA2 = float(np.log(1.25))   # GCNII layer-2 theta; layer 1 is dead code
SLOPE = 0.25
GSLOPE = 0.01                  # GAT leaky-relu slope
RG = [list(range(C))]

_CACHE = {}


def _build(reps=1):
    nc = bacc.Bacc("TRN2", target_bir_lowering=False, debug=False,
                   num_devices=C)
    d = {}
    d["xT_sl"] = nc.dram_tensor("xT_sl", [N, P], BF16, kind="ExternalInput")
    d["x_row"] = nc.dram_tensor("x_row", [N, N], BF16, kind="ExternalInput")
    d["adj_r"] = nc.dram_tensor("adj_r", [P, N], F32, kind="ExternalInput")
    d["Wg1"] = nc.dram_tensor("Wg1", [H1, N, N], BF16, kind="ExternalInput")
    d["Wa1"] = nc.dram_tensor("Wa1", [N, 2 * H1], BF16, kind="ExternalInput")
    d["Wo1"] = nc.dram_tensor("Wo1", [H1 * N, NC1], BF16, kind="ExternalInput")
    d["ao1"] = nc.dram_tensor("ao1", [2 * NC1], BF16, kind="ExternalInput")
    d["Wg2"] = nc.dram_tensor("Wg2", [H2, NC1, NC1], BF16, kind="ExternalInput")
    d["Wa2"] = nc.dram_tensor("Wa2", [NC1, 2 * H2], BF16, kind="ExternalInput")
    d["Wo2"] = nc.dram_tensor("Wo2", [N, N], BF16, kind="ExternalInput")
    d["Woa2"] = nc.dram_tensor("Woa2", [N, 2], BF16, kind="ExternalInput")
    d["fc0_w"] = nc.dram_tensor("fc0_w", [N, HID], BF16, kind="ExternalInput")
    d["fc0_b"] = nc.dram_tensor("fc0_b", [HID], BF16, kind="ExternalInput")
    d["fc1_w"] = nc.dram_tensor("fc1_w", [HID, N], BF16, kind="ExternalInput")
    d["fc1_b"] = nc.dram_tensor("fc1_b", [N], BF16, kind="ExternalInput")
    d["cw1T_sl"] = nc.dram_tensor("cw1T_sl", [N, P], BF16, kind="ExternalInput")
    out_d = nc.dram_tensor("out", [P, N], F32, kind="ExternalOutput")
    dbg = {}
    if os.environ.get("KDEBUG"):
        for nm, shp, dt in [("d_h0f", [P, HID], F32), ("d_uv1", [P, 2 * H1], F32),
                            ("d_n0", [P, N], BF16),
                            ("d_z0", [P, N], BF16), ("d_o0", [P, N], BF16),
                            ("d_wh", [P, NC1], BF16), ("d_uvo1", [P, 2], F32),
                            ("d_xg", [P, NC1 + 8], BF16),
                            ("d_uv2", [P, 2 * H2], F32),
                            ("d_hc2", [P, N + 16], BF16),
                            ("d_n20", [P, N], BF16), ("d_z20", [P, NC1], BF16),
                            ("d_xg2", [P, N], BF16), ("d_sf", [P, HID], F32)]:
            dbg[nm] = nc.dram_tensor(nm, shp, dt, kind="ExternalOutput")

    with tile.TileContext(nc) as tc:
        _body(nc, tc, d, out_d, reps, dbg)
    nc.compile()
    return nc


def _body(nc, tc, d, out_d, reps=1, dbg=None):
    dbg = dbg or {}

    def dtap(nm, ap):
        if nm in dbg:
            nc.sync.dma_start(dbg[nm].ap(), ap)

    with (
        tc.tile_pool(name="cst", bufs=1) as cst,          # constants
        tc.tile_pool(name="per", bufs=1) as per,          # cross-phase persistents
        tc.tile_pool(name="wstr", bufs=2) as w_str,       # big weight stream
        tc.tile_pool(name="full", bufs=2) as full_p,      # gathered full tensors
        tc.tile_pool(name="att", bufs=2) as att_p,        # per-layer attention
        tc.tile_pool(name="scbf", bufs=2) as sc_bf,       # bf16 scratch
        tc.tile_pool(name="sc32", bufs=2) as sc_32,       # f32 scratch
        tc.tile_pool(name="sm", bufs=2) as sm,            # small vectors
        tc.tile_pool(name="psbig", bufs=2, space="PSUM") as ps_big,   # [128,1024] f32
        tc.tile_pool(name="pswh", bufs=1, space="PSUM") as ps_wh,     # [128,512] f32
        tc.tile_pool(name="pstr", bufs=2, space="PSUM") as ps_tr,     # [128,128] bf16
        tc.tile_pool(name="psuv", bufs=1, space="PSUM") as ps_uv,     # tiny f32
        tc.tile_pool(name="dram", bufs=1, space="DRAM") as dram,
    ):
        ident = cst.tile([P, P], BF16, tag="ident")
        make_identity(nc, ident)

        # ---------- constant / persistent loads ----------
        xT_sb = per.tile([P, C, P], BF16, tag="xT")
        nc.scalar.dma_start(xT_sb[:], d["xT_sl"].ap().rearrange("(c p) m -> p c m", p=P))
        x_row = per.tile([P, C, N], BF16, tag="x_row")
        nc.scalar.dma_start(x_row[:], d["x_row"].ap().rearrange("(c p) f -> p c f", p=P))
        wa1_sb = cst.tile([P, C, 2 * H1], BF16, tag="wa1")
        nc.scalar.dma_start(wa1_sb[:], d["Wa1"].ap().rearrange("(c p) f -> p c f", p=P))
        fc0_sb = cst.tile([P, C, HID], BF16, tag="fc0")
        nc.scalar.dma_start(fc0_sb[:], d["fc0_w"].ap().rearrange("(c p) f -> p c f", p=P))
        wa2_sb = cst.tile([P, 4, 2 * H2], BF16, tag="wa2")
        nc.scalar.dma_start(wa2_sb[:], d["Wa2"].ap().rearrange("(c p) f -> p c f", p=P))
        cw1T_sb = cst.tile([P, C, P], BF16, tag="cw1T")
        nc.scalar.dma_start(cw1T_sb[:], d["cw1T_sl"].ap().rearrange("(c p) m -> p c m", p=P))

        adj_sb = sc_32.tile([P, N], F32, tag="s32")
        nc.scalar.dma_start(adj_sb[:], d["adj_r"].ap())
        madj = per.tile([P, N], BF16, tag="madj")        # 0 where adj>0 else -9e15
        nc.vector.tensor_scalar(madj[:], adj_sb[:], 0.0, None, op0=OP.is_gt)
        nc.vector.tensor_scalar(madj[:], madj[:], 1.0, 9e15,
                                op0=OP.subtract, op1=OP.mult)

        def bcast_const(L, src_ap, tag):
            row = sm.tile([1, N], BF16, tag="vrow", bufs=1)
            nc.gpsimd.dma_start(row[:1, :L], src_ap[None, :])
            bc = cst.tile([P, L], BF16, tag=f"bc_{tag}")
            nc.gpsimd.partition_broadcast(bc[:], row[:1, :L])
            return bc

        ao1_bc = bcast_const(2 * NC1, d["ao1"].ap(), "ao1")
        woa2u_bc = bcast_const(N, d["Woa2"].ap()[:, 0], "w2u")
        woa2v_bc = bcast_const(N, d["Woa2"].ap()[:, 1], "w2v")
        fc0b_bc = bcast_const(HID, d["fc0_b"].ap(), "f0b")
        fc1b_bc = bcast_const(N, d["fc1_b"].ap(), "f1b")

        # ---------------- helpers ----------------
        def allgather(src_sb, rows, cols, tag):
            ag_in = dram.tile([rows, cols], BF16, tag=f"agi_{tag}")
            ag_out = dram.tile([C * rows, cols], BF16, tag=f"ago_{tag}",
                               addr_space="Shared")
            nc.gpsimd.dma_start(ag_in[:], src_sb)
            nc.gpsimd.collective_compute(
                "AllGather", OP.bypass, replica_groups=RG,
                ins=[ag_in.opt()], outs=[ag_out.opt()])
            return ag_out

        def dma_T(src_bf_2d, dst_3d, eng):
            """[128, k*128] -> [128, k, 128] chunked transpose via DMA XBAR."""
            eng.dma_start_transpose(dst_3d, src_bf_2d)

        def vb_broadcast(row_src_3d):
            """[1, C, 128] DRAM view -> [1,N] sbuf -> [128,N]."""
            vrow = sm.tile([1, N], BF16, tag="vrow", bufs=1)
            nc.sync.dma_start(vrow[:1].rearrange("o (c p) -> o c p", p=P),
                              row_src_3d)
            vb = att_p.tile([P, N], BF16, tag="vb", bufs=2)
            nc.gpsimd.partition_broadcast(vb[:], vrow[:1, :])
            return vb

        def col_extract_vb(full_3d_col, tag):
            """[128, C] column view of a gathered payload -> vb [128, N].

            transpose (PE, tiny) -> sbuf -> DRAM bounce -> broadcast."""
            tp = ps_tr.tile([P, P], BF16, tag="tr")
            nc.tensor.transpose(tp[:C, :P], full_3d_col, ident[:])
            v_sb = sm.tile([C, P], BF16, tag="vx", bufs=2)
            nc.vector.tensor_copy(v_sb[:], tp[:C, :P])
            v_dr = dram.tile([C, P], BF16, tag=f"vxd_{tag}")
            nc.sync.dma_start(v_dr[:], v_sb[:])
            return vb_broadcast(v_dr[:][None])

        def softmax_rows(u_ap, vb_ap, tagid):
            """n_bf, rs = exp(lrelu(u + v^T) masked), 1/rowsum."""
            e_bf = sc_bf.tile([P, N], BF16, tag="ebf")
            nc.vector.scalar_tensor_tensor(e_bf[:], vb_ap, u_ap, madj[:],
                                           op0=OP.add, op1=OP.add)
            nc.vector.scalar_tensor_tensor(e_bf[:], e_bf[:], GSLOPE, e_bf[:],
                                           op0=OP.mult, op1=OP.max)
            n_bf = att_p.tile([P, N], BF16, tag="nbf")
            ssum = sm.tile([P, 1], F32, tag=f"ss_{tagid}")
            nc.scalar.activation(n_bf[:], e_bf[:], AF.Exp, accum_out=ssum[:])
            rs = sm.tile([P, 1], F32, tag=f"rs_{tagid}")
            nc.vector.reciprocal(rs[:], ssum[:])
            return n_bf, rs

        def elu_store(o_ps, dst_bf, L, rs=None):
            """dst = elu(rs * o_ps); rs=None means already scaled."""
            m32 = sc_32.tile([P, N], F32, tag="s32")
            r32 = sc_32.tile([P, N], F32, tag="s32c", bufs=1)
            if rs is not None:
                nc.vector.tensor_scalar(m32[:, :L], o_ps, rs[:], 0.0,
                                        op0=OP.mult, op1=OP.min)
                nc.vector.tensor_scalar(r32[:, :L], o_ps, rs[:], 0.0,
                                        op0=OP.mult, op1=OP.max)
            else:
                nc.vector.tensor_scalar(m32[:, :L], o_ps, 0.0, None, op0=OP.min)
                nc.vector.tensor_scalar(r32[:, :L], o_ps, 0.0, None, op0=OP.max)
            g32 = sc_32.tile([P, N], F32, tag="s32b", bufs=1)
            nc.scalar.activation(g32[:, :L], m32[:, :L], AF.Exp)
            nc.vector.scalar_tensor_tensor(dst_bf, g32[:, :L], -1.0, r32[:, :L],
                                           op0=OP.add, op1=OP.add)

        # persistent per-rep tensors
        hcatT = per.tile([P, H1 * C, P], BF16, tag="hcatT")   # [128, 40, 128]
        h0f = per.tile([P, HID], F32, tag="h0f")
        uv1_sb = per.tile([P, 2 * H1], F32, tag="uv1")
        h0_full = per.tile([P, C, HID], BF16, tag="h0full")

        wg_view = [d["Wg1"].ap()[h].rearrange("(c p) f -> p c f", p=P)
                   for h in range(H1)]
        wo1_view = d["Wo1"].ap().rearrange("(g c p) f -> p g c f", p=P, c=C)
        wg2_view = [d["Wg2"].ap()[h].rearrange("(c p) f -> p c f", p=P)
                    for h in range(H2)]
        wo2_view = d["Wo2"].ap().rearrange("(c p) f -> p c f", p=P)
        fc1_view = d["fc1_w"].ap().rearrange("(c p) f -> p c f", p=P)

        for _rep in range(reps):
            # ======== GCNII h0 = lrelu(x@fc0 + b) ========
            h0_ps = ps_wh.tile([P, HID], F32, tag="wh")
            for c in range(C):
                nc.tensor.matmul(h0_ps[:], xT_sb[:, c, :], fc0_sb[:, c, :],
                                 start=(c == 0), stop=(c == C - 1))
            nc.vector.scalar_tensor_tensor(h0f[:], h0_ps[:], 1.0, fc0b_bc[:],
                                           op0=OP.mult, op1=OP.add)
            nc.vector.scalar_tensor_tensor(h0f[:], h0f[:], SLOPE, h0f[:],
                                           op0=OP.mult, op1=OP.max)
            h0b = sc_bf.tile([P, HID], BF16, tag="h0b")
            nc.vector.tensor_copy(h0b[:], h0f[:])
            dtap("d_h0f", h0f[:])

            # ======== GAT1 u,v for all heads: uv = x @ Wa1 ========
            uv1_ps = ps_uv.tile([P, 2 * H1], F32, tag="uv")
            for c in range(C):
                nc.tensor.matmul(uv1_ps[:], xT_sb[:, c, :], wa1_sb[:, c, :],
                                 start=(c == 0), stop=(c == C - 1))
            nc.vector.tensor_copy(uv1_sb[:], uv1_ps[:])
            dtap("d_uv1", uv1_sb[:])
            # v rows (cols H1..2H1) -> [5,128] for the tiny AG
            v1_bf = sc_bf.tile([P, 2 * H1], BF16, tag="v1bf")
            nc.vector.tensor_copy(v1_bf[:], uv1_sb[:])
            vtr_ps = ps_tr.tile([P, P], BF16, tag="tr")
            nc.tensor.transpose(vtr_ps[:2 * H1, :P], v1_bf[:], ident[:])
            vtr_sb = sm.tile([2 * H1, P], BF16, tag="vtr", bufs=1)
            nc.vector.tensor_copy(vtr_sb[:], vtr_ps[:2 * H1, :P])
            ag_v1 = allgather(vtr_sb[H1:2 * H1, :], H1, P, "v1")
            v1_rows = ag_v1[:].rearrange("(c h) p -> h c p", h=H1)

            # GAT1 weight stream: heads 0,1 prefetch now
            wg_sb = []
            for h in range(2):
                t = w_str.tile([P, C, N], BF16, tag="wstream")
                nc.scalar.dma_start(t[:], wg_view[h])
                wg_sb.append(t)
            wo1_sb = []
            t = w_str.tile([P, C, NC1], BF16, tag="wo1stream")
            nc.scalar.dma_start(t[:], wo1_view[:, 0])
            wo1_sb.append(t)

            wh_ps = ps_wh.tile([P, NC1], F32, tag="wh")   # o1 Wh accumulator

            # ======== GAT1: 5 heads, software-pipelined ========
            vbs = {0: vb_broadcast(v1_rows[0][None]),
                   1: vb_broadcast(v1_rows[1][None])}
            sm_state = {0: softmax_rows(uv1_sb[:, 0:1], vbs.pop(0)[:], "g1")}  # noqa
            attTs = {0: att_p.tile([P, C, P], BF16, tag="attT", name="attT0")}
            dma_T(sm_state[0][0][:], attTs[0][:], nc.sync)
            dtap("d_n0", sm_state[0][0][:])
            for h in range(H1):
                n_bf, rs = sm_state.pop(h)
                attT = attTs.pop(h)
                # z = att @ x_full
                z_ps = ps_big.tile([P, N], F32, tag="big")
                for j in range(C):
                    for s in range(2):
                        nc.tensor.matmul(z_ps[:, s * 512:(s + 1) * 512],
                                         attT[:, j, :], x_row[:, j, s * 512:(s + 1) * 512],
                                         start=(j == 0), stop=(j == C - 1))
                z_bf = sc_bf.tile([P, N], BF16, tag="zbf")
                nc.vector.tensor_scalar(z_bf[:], z_ps[:], rs[:], None, op0=OP.mult)
                if h == 0:
                    dtap("d_z0", z_bf[:])
                zT = att_p.tile([P, C, P], BF16, tag="zT")
                dma_T(z_bf[:], zT[:], nc.sync)
                # o1-Wh accumulation chunks of the PREVIOUS head (fills the
                # z->o latency window on the PE)
                if h > 0:
                    for j in range(C):
                        nc.tensor.matmul(wh_ps[:], hcatT[:, (h - 1) * C + j, :],
                                         wo1_sb[h - 1][:, j, :],
                                         start=(h == 1 and j == 0), stop=False,
                                         skip_group_check=True)
                # next head's softmax + attT transpose (overlaps PE work)
                if h + 2 < H1:
                    vbs[h + 2] = vb_broadcast(v1_rows[h + 2][None])
                if h + 1 < H1:
                    sm_state[h + 1] = softmax_rows(uv1_sb[:, h + 1:h + 2],
                                                   vbs.pop(h + 1)[:], "g1")
                    attTs[h + 1] = att_p.tile([P, C, P], BF16, tag="attT",
                                              name=f"attT{h + 1}")
                    dma_T(sm_state[h + 1][0][:], attTs[h + 1][:], nc.sync)
                # out = z @ Wg1[h]
                o_ps = ps_big.tile([P, N], F32, tag="big")
                wgh = wg_sb[h]
                for j in range(C):
                    for s in range(2):
                        nc.tensor.matmul(o_ps[:, s * 512:(s + 1) * 512],
                                         zT[:, j, :], wgh[:, j, s * 512:(s + 1) * 512],
                                         start=(j == 0), stop=(j == C - 1))
                # weight prefetches
                if h + 2 < H1:
                    t = w_str.tile([P, C, N], BF16, tag="wstream")
                    nc.scalar.dma_start(t[:], wg_view[h + 2])
                    wg_sb.append(t)
                if h + 1 < H1:
                    t = w_str.tile([P, C, NC1], BF16, tag="wo1stream")
                    nc.scalar.dma_start(t[:], wo1_view[:, h + 1])
                    wo1_sb.append(t)
                o_bf = sc_bf.tile([P, N], BF16, tag="obf")
                elu_store(o_ps[:], o_bf[:], N)
                if h == 0:
                    dtap("d_o0", o_bf[:])
                dma_T(o_bf[:], hcatT[:, h * C:(h + 1) * C, :], nc.scalar)
            # last head's o1-Wh chunks
            for j in range(C):
                nc.tensor.matmul(wh_ps[:], hcatT[:, (H1 - 1) * C + j, :],
                                 wo1_sb[H1 - 1][:, j, :],
                                 start=False, stop=(j == C - 1),
                                 skip_group_check=True)

            # h0 allgather (result needed only in GCNII tail)
            ag_h0 = allgather(h0b[:], P, HID, "h0")
            nc.gpsimd.dma_start(h0_full[:], ag_h0[:].rearrange("(c p) f -> p c f", p=P))

            # ======== GAT1 out-attention (o1) ========
            junk = sc_bf.tile([P, N], BF16, tag="zbf")
            uvo1 = sm.tile([P, 2], F32, tag="uvo1")
            nc.vector.scalar_tensor_tensor(junk[:, :NC1], wh_ps[:], 1.0,
                                           ao1_bc[:, :NC1], op0=OP.mult,
                                           op1=OP.mult, accum_out=uvo1[:, 0:1])
            nc.vector.scalar_tensor_tensor(junk[:, :NC1], wh_ps[:], 1.0,
                                           ao1_bc[:, NC1:], op0=OP.mult,
                                           op1=OP.mult, accum_out=uvo1[:, 1:2])
            dtap("d_uvo1", uvo1[:])
            # payload [Wh | v | pad]
            pay_wh = sc_bf.tile([P, NC1 + 8], BF16, tag="pay520")
            nc.vector.tensor_copy(pay_wh[:, :NC1], wh_ps[:])
            nc.vector.tensor_copy(pay_wh[:, NC1:NC1 + 1], uvo1[:, 1:2])
            nc.vector.memset(pay_wh[:, NC1 + 1:], 0.0)
            dtap("d_wh", pay_wh[:, :NC1])
            ag_wh = allgather(pay_wh[:], P, NC1 + 8, "wh")
            wh_full = full_p.tile([P, C, NC1 + 8], BF16, tag="full520")
            nc.gpsimd.dma_start(wh_full[:], ag_wh[:].rearrange("(c p) f -> p c f", p=P))
            vb = col_extract_vb(wh_full[:, :, NC1], "o1")
            n_bf, rs = softmax_rows(uvo1[:, 0:1], vb[:], "o1")
            attT = att_p.tile([P, C, P], BF16, tag="attT")
            dma_T(n_bf[:], attT[:], nc.sync)
            xg_ps = ps_wh.tile([P, NC1], F32, tag="wh")
            for j in range(C):
                nc.tensor.matmul(xg_ps[:], attT[:, j, :], wh_full[:, j, :NC1],
                                 start=(j == 0), stop=(j == C - 1))
            # xg = elu(rs * xg_ps) -> payload [xg | v1 v2 | pad]
            pay_g2 = sc_bf.tile([P, NC1 + 8], BF16, tag="pay520")
            elu_store(xg_ps[:], pay_g2[:, :NC1], NC1, rs=rs)
            xgT = att_p.tile([P, 4, P], BF16, tag="xgT")
            dma_T(pay_g2[:, :NC1], xgT[:], nc.scalar)
            uv2_ps = ps_uv.tile([P, 2 * H2], F32, tag="uv")
            for c in range(4):
                nc.tensor.matmul(uv2_ps[:], xgT[:, c, :], wa2_sb[:, c, :],
                                 start=(c == 0), stop=(c == 3))
            uv2_sb = sm.tile([P, 2 * H2], F32, tag="uv2")
            nc.vector.tensor_copy(uv2_sb[:], uv2_ps[:])
            nc.vector.tensor_copy(pay_g2[:, NC1:NC1 + 2], uv2_sb[:, H2:])
            nc.vector.memset(pay_g2[:, NC1 + 2:], 0.0)
            dtap("d_xg", pay_g2[:])
            dtap("d_uv2", uv2_sb[:])
            ag_xg = allgather(pay_g2[:], P, NC1 + 8, "xg")
            xg_full = full_p.tile([P, C, NC1 + 8], BF16, tag="full520")
            nc.gpsimd.dma_start(xg_full[:], ag_xg[:].rearrange("(c p) f -> p c f", p=P))
            # wg2 stream (needed from here on)
            wg2_sb = w_str.tile([P, H2, 4, NC1], BF16, tag="wo1stream")
            for h in range(H2):
                nc.scalar.dma_start(wg2_sb[:, h], wg2_view[h])

            # ======== GAT2: 2 heads (pipelined) ========
            pay_o2 = sc_bf.tile([P, N + 16], BF16, tag="payo2", bufs=1)
            vbs2 = {h: col_extract_vb(xg_full[:, :, NC1 + h], f"g2_{h}")
                    for h in range(H2)}
            sm2 = {0: softmax_rows(uv2_sb[:, 0:1], vbs2.pop(0)[:], "g2")}
            attT2 = {0: att_p.tile([P, C, P], BF16, tag="attT", name="attT20")}
            dma_T(sm2[0][0][:], attT2[0][:], nc.sync)
            for h in range(H2):
                n_bf, rs = sm2.pop(h)
                if h == 0:
                    dtap("d_n20", n_bf[:])
                attT = attT2.pop(h)
                z_ps = ps_wh.tile([P, NC1], F32, tag="wh")
                for j in range(C):
                    nc.tensor.matmul(z_ps[:], attT[:, j, :],
                                     xg_full[:, j, :NC1],
                                     start=(j == 0), stop=(j == C - 1))
                z_bf = sc_bf.tile([P, NC1], BF16, tag="h0b")
                nc.vector.tensor_scalar(z_bf[:], z_ps[:], rs[:], None, op0=OP.mult)
                if h == 0:
                    dtap("d_z20", z_bf[:])
                zT = att_p.tile([P, 4, P], BF16, tag="xgT")
                dma_T(z_bf[:], zT[:], nc.sync)
                if h + 1 < H2:
                    sm2[h + 1] = softmax_rows(uv2_sb[:, h + 1:h + 2],
                                              vbs2.pop(h + 1)[:], "g2")
                    attT2[h + 1] = att_p.tile([P, C, P], BF16, tag="attT",
                                               name=f"attT2{h + 1}")
                    dma_T(sm2[h + 1][0][:], attT2[h + 1][:], nc.sync)
                o_ps = ps_wh.tile([P, NC1], F32, tag="wh")
                for j in range(4):
                    nc.tensor.matmul(o_ps[:], zT[:, j, :], wg2_sb[:, h, j, :],
                                     start=(j == 0), stop=(j == 3))
                elu_store(o_ps[:], pay_o2[:, h * NC1:(h + 1) * NC1], NC1)

            # ======== GAT2 out-attention (o2) ========
            junk2 = sc_bf.tile([P, N], BF16, tag="zbf")
            uvo2 = sm.tile([P, 2], F32, tag="uvo2")
            nc.vector.scalar_tensor_tensor(junk2[:], pay_o2[:, :N], 1.0,
                                           woa2u_bc[:], op0=OP.mult,
                                           op1=OP.mult, accum_out=uvo2[:, 0:1])
            nc.vector.scalar_tensor_tensor(junk2[:], pay_o2[:, :N], 1.0,
                                           woa2v_bc[:], op0=OP.mult,
                                           op1=OP.mult, accum_out=uvo2[:, 1:2])
            nc.vector.tensor_copy(pay_o2[:, N:N + 1], uvo2[:, 1:2])
            nc.vector.memset(pay_o2[:, N + 1:], 0.0)
            dtap("d_hc2", pay_o2[:])
            ag_h2 = allgather(pay_o2[:], P, N + 16, "h2")
            h2_full = full_p.tile([P, C, N + 16], BF16, tag="h2full", bufs=1)
            nc.gpsimd.dma_start(h2_full[:], ag_h2[:].rearrange("(c p) f -> p c f", p=P))
            # Wo2 stream (during the AG)
            wo2_sb = w_str.tile([P, C, N], BF16, tag="wstream")
            nc.scalar.dma_start(wo2_sb[:], wo2_view)
            vb = col_extract_vb(h2_full[:, :, N], "o2")
            n_bf, rs = softmax_rows(uvo2[:, 0:1], vb[:], "o2")
            attT = att_p.tile([P, C, P], BF16, tag="attT")
            dma_T(n_bf[:], attT[:], nc.sync)
            # z = att @ hcat2_full
            z_ps = ps_big.tile([P, N], F32, tag="big")
            for j in range(C):
                for s in range(2):
                    nc.tensor.matmul(z_ps[:, s * 512:(s + 1) * 512],
                                     attT[:, j, :],
                                     h2_full[:, j, s * 512:(s + 1) * 512],
                                     start=(j == 0), stop=(j == C - 1))
            z_bf = sc_bf.tile([P, N], BF16, tag="zbf")
            nc.vector.tensor_scalar(z_bf[:], z_ps[:], rs[:], None, op0=OP.mult)
            zT = att_p.tile([P, C, P], BF16, tag="zT")
            dma_T(z_bf[:], zT[:], nc.sync)
            o_ps = ps_big.tile([P, N], F32, tag="big")
            for j in range(C):
                for s in range(2):
                    nc.tensor.matmul(o_ps[:, s * 512:(s + 1) * 512],
                                     zT[:, j, :], wo2_sb[:, j, s * 512:(s + 1) * 512],
                                     start=(j == 0), stop=(j == C - 1))
            xg2_bf = sc_bf.tile([P, N], BF16, tag="obf")
            elu_store(o_ps[:], xg2_bf[:], N)
            dtap("d_xg2", xg2_bf[:])
            xg2T = att_p.tile([P, C, P], BF16, tag="zT")
            dma_T(xg2_bf[:], xg2T[:], nc.sync)

            # ======== GCNII ========
            hi_ps = ps_wh.tile([P, HID], F32, tag="wh")
            for j in range(C):
                nc.tensor.matmul(hi_ps[:], xg2T[:, j, :], h0_full[:, j, :],
                                 start=(j == 0), stop=(j == C - 1))
            sf = sc_32.tile([P, HID], F32, tag="sf", bufs=1)
            nc.vector.scalar_tensor_tensor(sf[:], hi_ps[:], 9.0, h0f[:],
                                           op0=OP.mult, op1=OP.add)
            nc.vector.tensor_scalar(sf[:], sf[:], 0.1, None, op0=OP.mult)
            s_bf = sc_bf.tile([P, HID], BF16, tag="h0b")
            nc.vector.tensor_copy(s_bf[:], sf[:])
            dtap("d_sf", sf[:])
            ag_s = allgather(s_bf[:], P, HID, "s")
            s_full = full_p.tile([P, C, HID], BF16, tag="sfull", bufs=1)
            nc.gpsimd.dma_start(s_full[:], ag_s[:].rearrange("(c p) f -> p c f", p=P))
            fc1_sb = w_str.tile([P, 4, N], BF16, tag="wo1stream")
            nc.scalar.dma_start(fc1_sb[:], fc1_view)
            mm_ps = ps_wh.tile([P, HID], F32, tag="wh")
            for c in range(C):
                nc.tensor.matmul(mm_ps[:], cw1T_sb[:, c, :], s_full[:, c, :],
                                 start=(c == 0), stop=(c == C - 1))
            hf = sc_32.tile([P, HID], F32, tag="s32")
            nc.vector.scalar_tensor_tensor(hf[:], sf[:], (1.0 - THETA2) / THETA2,
                                           mm_ps[:], op0=OP.mult, op1=OP.add)
            nc.vector.scalar_tensor_tensor(hf[:], hf[:], THETA2, h0f[:],
                                           op0=OP.mult, op1=OP.add)
            nc.vector.scalar_tensor_tensor(hf[:], hf[:], SLOPE, hf[:],
                                           op0=OP.mult, op1=OP.max)
            hb = sc_bf.tile([P, HID], BF16, tag="h0b")
            nc.vector.tensor_copy(hb[:], hf[:])
            hT = att_p.tile([P, 4, P], BF16, tag="xgT")
            dma_T(hb[:], hT[:], nc.scalar)
            y_ps = ps_big.tile([P, N], F32, tag="big")
            for c in range(4):
                for s in range(2):
                    nc.tensor.matmul(y_ps[:, s * 512:(s + 1) * 512], hT[:, c, :],
                                     fc1_sb[:, c, s * 512:(s + 1) * 512],
                                     start=(c == 0), stop=(c == 3))
            y_sb = sc_32.tile([P, N], F32, tag="s32")
            nc.vector.scalar_tensor_tensor(y_sb[:], y_ps[:], 1.0, fc1b_bc[:],
                                           op0=OP.mult, op1=OP.add)
            nc.sync.dma_start(out_d.ap(), y_sb[:])


def _shard_inputs(inputs):
    f32 = lambda a: np.asarray(a, dtype=np.float32)
    bf = lambda a: np.ascontiguousarray(f32(a)).astype(ml_dtypes.bfloat16)
    x = f32(inputs["x"])
    adj = f32(inputs["adj"])
    x_bf = bf(x)
    xT_bf = np.ascontiguousarray(x_bf.T)
    cw1T = np.ascontiguousarray(bf(inputs["cw1"]).T)
    Wg1 = f32(inputs["Wg1"])
    ag1 = f32(inputs["ag1"])[:, :, 0]          # [5, 2048]
    Wa1 = np.stack([Wg1[h] @ ag1[h, :N] for h in range(H1)] +
                   [Wg1[h] @ ag1[h, N:] for h in range(H1)], axis=1)  # [1024, 10]
    Wg2 = f32(inputs["Wg2"])
    ag2 = f32(inputs["ag2"])[:, :, 0]          # [2, 1024]
    Wa2 = np.stack([Wg2[h] @ ag2[h, :NC1] for h in range(H2)] +
                   [Wg2[h] @ ag2[h, NC1:] for h in range(H2)], axis=1)  # [512, 4]
    Wo2 = f32(inputs["Wo2"])
    ao2 = f32(inputs["ao2"])[:, 0]             # [2048]
    Woa2 = np.stack([Wo2 @ ao2[:N], Wo2 @ ao2[N:]], axis=1)  # [1024, 2]
    shared = {
        "x_row": x_bf,
        "Wg1": bf(Wg1),
        "Wa1": bf(Wa1),
        "Wo1": bf(inputs["Wo1"]),
        "ao1": bf(f32(inputs["ao1"])[:, 0]),
        "Wg2": bf(Wg2),
        "Wa2": bf(Wa2),
        "Wo2": bf(Wo2),
        "Woa2": bf(Woa2),
        "fc0_w": bf(inputs["fc0_w"]),
        "fc0_b": bf(inputs["fc0_b"]),
        "fc1_w": bf(inputs["fc1_w"]),
        "fc1_b": bf(inputs["fc1_b"]),
    }
    in_maps = []
    for c in range(C):
        r0, r1 = c * P, (c + 1) * P
        m = dict(shared)
        m["xT_sl"] = np.ascontiguousarray(xT_bf[:, r0:r1])
        m["adj_r"] = np.ascontiguousarray(adj[r0:r1])
        m["cw1T_sl"] = np.ascontiguousarray(cw1T[:, r0:r1])
        in_maps.append(m)
    return in_maps


def kernel(**inputs) -> np.ndarray:
    if "nc" not in _CACHE:
        _CACHE["nc"] = _build()
    nc = _CACHE["nc"]
    in_maps = _shard_inputs(inputs)
    res = run_bass_kernel_spmd(nc, in_maps, core_ids=list(range(C)))
    out = np.concatenate([res.results[c]["out"] for c in range(C)], axis=0)
    return np.asarray(out, dtype=np.float32)


# revision 19
# speedup vs baseline: 1.3503x; 1.2339x over previous
"""Trainium2 Bass kernel for nn_GCNII_80178449482260 (2x dense GAT + GCNII).

Row-parallel over N=1024 across 8 cores (128 rows each), restructured to
minimize collectives and keep the PE continuously busy (p-state ramp):

  * GAT layers use  att @ Wh == (att @ h_in) @ W  and
    u,v == h_in @ (W @ a_half)  with W@a precomputed on host.  Since x is
    replicated, the five GAT1 heads need NO big all-gather -- just one tiny
    [5,128] v-gather.  GAT2's two heads share one input all-gather.
  * Collectives (6): v1(tiny), h0, [Wh_o1|v], [xg|v], [hcat2|v], support.
    All outputs in Shared DRAM; v vectors ride inside the payloads.
  * All [128, k*128] transposes via DMA XBAR (dma_start_transpose) on the
    two HWDGE rings (sync + scalar) -- zero PE/DVE cost.
  * Scalar engine runs ONLY Exp; lrelu/elu masks on DVE (bf16 where safe);
    PSUM->SBUF copies on DVE (gpsimd has no PSUM port).
  * Weights stream as single big DMAs (split across 16 SDMA engines).
  * Per-head software pipelining: head h+1's softmax chain and the o1-Wh
    accumulation of head h-1 are interleaved into head h's matmul stream.
"""
import os
import sys
import numpy as np

for _p in ("/opt/trn_rl_repo", "/root/.axon_site/_ro/trn_rl_repo"):
    if _p not in sys.path:
        sys.path.insert(0, _p)

import ml_dtypes  # noqa: E402
from concourse import bacc, tile, mybir  # noqa: E402
from concourse.bass_utils import run_bass_kernel_spmd  # noqa: E402
from concourse.kernels.tile_matmul import make_identity  # noqa: E402

BF16 = mybir.dt.bfloat16
F32 = mybir.dt.float32
AF = mybir.ActivationFunctionType
OP = mybir.AluOpType

N = 1024      # nodes
P = 128       # partitions / rows per core
C = 8         # cores
HID = 512
NC1 = 512
H1, H2 = 5, 2
THETA2 = float(np.log(1.25))   # GCNII layer-2 theta; layer 1 is dead code
SLOPE = 0.25
GSLOPE = 0.01                  # GAT leaky-relu slope
RG = [list(range(C))]

_CACHE = {}


def _build(reps=1):
    nc = bacc.Bacc("TRN2", target_bir_lowering=False, debug=False,
                   num_devices=C)
    d = {}
    d["xT_sl"] = nc.dram_tensor("xT_sl", [N, P], BF16, kind="ExternalInput")
    d["x_row"] = nc.dram_tensor("x_row", [N, N], BF16, kind="ExternalInput")
    d["adj_r"] = nc.dram_tensor("adj_r", [P, N], F32, kind="ExternalInput")
    d["Wg1"] = nc.dram_tensor("Wg1", [H1, N, N], BF16, kind="ExternalInput")
    d["Wa1"] = nc.dram_tensor("Wa1", [N, 2 * H1], BF16, kind="ExternalInput")
    d["Wo1"] = nc.dram_tensor("Wo1", [H1 * N, NC1], BF16, kind="ExternalInput")
    d["ao1"] = nc.dram_tensor("ao1", [2 * NC1], BF16, kind="ExternalInput")
    d["Wg2"] = nc.dram_tensor("Wg2", [H2, NC1, NC1], BF16, kind="ExternalInput")
    d["Wa2"] = nc.dram_tensor("Wa2", [NC1, 2 * H2], BF16, kind="ExternalInput")
    d["Wo2"] = nc.dram_tensor("Wo2", [N, N], BF16, kind="ExternalInput")
    d["Woa2"] = nc.dram_tensor("Woa2", [N, 2], BF16, kind="ExternalInput")
    d["fc0_w"] = nc.dram_tensor("fc0_w", [N, HID], BF16, kind="ExternalInput")
    d["fc0_b"] = nc.dram_tensor("fc0_b", [HID], BF16, kind="ExternalInput")
    d["fc1_w"] = nc.dram_tensor("fc1_w", [HID, N], BF16, kind="ExternalInput")
    d["fc1_b"] = nc.dram_tensor("fc1_b", [N], BF16, kind="ExternalInput")
    d["cw1T_sl"] = nc.dram_tensor("cw1T_sl", [N, P], BF16, kind="ExternalInput")
    out_d = nc.dram_tensor("out", [P, N], F32, kind="ExternalOutput")
    dbg = {}
    if os.environ.get("KDEBUG"):
        for nm, shp, dt in [("d_h0f", [P, HID], F32), ("d_uv1", [P, 2 * H1], F32),
                            ("d_n0", [P, N], BF16),
                            ("d_z0", [P, N], BF16), ("d_o0", [P, N], BF16),
                            ("d_wh", [P, NC1], BF16), ("d_uvo1", [P, 2], F32),
                            ("d_xg", [P, NC1 + 8], BF16),
                            ("d_uv2", [P, 2 * H2], F32),
                            ("d_hc2", [P, N + 16], BF16),
                            ("d_n20", [P, N], BF16), ("d_z20", [P, NC1], BF16),
                            ("d_xg2", [P, N], BF16), ("d_sf", [P, HID], F32)]:
            dbg[nm] = nc.dram_tensor(nm, shp, dt, kind="ExternalOutput")

    with tile.TileContext(nc) as tc:
        _body(nc, tc, d, out_d, reps, dbg)
    nc.compile()
    return nc


def _body(nc, tc, d, out_d, reps=1, dbg=None):
    dbg = dbg or {}

    def dtap(nm, ap):
        if nm in dbg:
            nc.sync.dma_start(dbg[nm].ap(), ap)

    with (
        tc.tile_pool(name="cst", bufs=1) as cst,          # constants
        tc.tile_pool(name="per", bufs=1) as per,          # cross-phase persistents
        tc.tile_pool(name="wstr", bufs=2) as w_str,       # big weight stream
        tc.tile_pool(name="full", bufs=2) as full_p,      # gathered full tensors
        tc.tile_pool(name="att", bufs=2) as att_p,        # per-layer attention
        tc.tile_pool(name="scbf", bufs=2) as sc_bf,       # bf16 scratch
        tc.tile_pool(name="sc32", bufs=2) as sc_32,       # f32 scratch
        tc.tile_pool(name="sm", bufs=2) as sm,            # small vectors
        tc.tile_pool(name="psbig", bufs=2, space="PSUM") as ps_big,   # [128,1024] f32
        tc.tile_pool(name="pswh", bufs=1, space="PSUM") as ps_wh,     # [128,512] f32
        tc.tile_pool(name="pstr", bufs=2, space="PSUM") as ps_tr,     # [128,1024] bf16
        tc.tile_pool(name="psuv", bufs=1, space="PSUM") as ps_uv,     # tiny f32
        tc.tile_pool(name="dram", bufs=1, space="DRAM") as dram,
    ):
        ident = cst.tile([P, P], BF16, tag="ident")
        make_identity(nc, ident)

        # ---------- constant / persistent loads ----------
        xT_sb = per.tile([P, C, P], BF16, tag="xT")
        nc.scalar.dma_start(xT_sb[:], d["xT_sl"].ap().rearrange("(c p) m -> p c m", p=P))
        x_row = per.tile([P, C, N], BF16, tag="x_row")
        nc.scalar.dma_start(x_row[:], d["x_row"].ap().rearrange("(c p) f -> p c f", p=P))
        wa1_sb = cst.tile([P, C, 2 * H1], BF16, tag="wa1")
        nc.scalar.dma_start(wa1_sb[:], d["Wa1"].ap().rearrange("(c p) f -> p c f", p=P))
        fc0_sb = cst.tile([P, C, HID], BF16, tag="fc0")
        nc.scalar.dma_start(fc0_sb[:], d["fc0_w"].ap().rearrange("(c p) f -> p c f", p=P))
        wa2_sb = cst.tile([P, 4, 2 * H2], BF16, tag="wa2")
        nc.scalar.dma_start(wa2_sb[:], d["Wa2"].ap().rearrange("(c p) f -> p c f", p=P))
        cw1T_sb = cst.tile([P, C, P], BF16, tag="cw1T")
        nc.scalar.dma_start(cw1T_sb[:], d["cw1T_sl"].ap().rearrange("(c p) m -> p c m", p=P))

        adj_sb = sc_32.tile([P, N], F32, tag="s32")
        nc.scalar.dma_start(adj_sb[:], d["adj_r"].ap())
        madj = per.tile([P, N], BF16, tag="madj")        # 0 where adj>0 else -9e15
        nc.vector.tensor_scalar(madj[:], adj_sb[:], 0.0, None, op0=OP.is_gt)
        nc.vector.tensor_scalar(madj[:], madj[:], 1.0, 9e15,
                                op0=OP.subtract, op1=OP.mult)

        def bcast_const(L, src_ap, tag):
            row = sm.tile([1, N], BF16, tag="vrow", bufs=1)
            nc.gpsimd.dma_start(row[:1, :L], src_ap[None, :])
            bc = cst.tile([P, L], BF16, tag=f"bc_{tag}")
            nc.gpsimd.partition_broadcast(bc[:], row[:1, :L])
            return bc

        ao1_bc = bcast_const(2 * NC1, d["ao1"].ap(), "ao1")
        woa2u_bc = bcast_const(N, d["Woa2"].ap()[:, 0], "w2u")
        woa2v_bc = bcast_const(N, d["Woa2"].ap()[:, 1], "w2v")
        fc0b_bc = bcast_const(HID, d["fc0_b"].ap(), "f0b")
        fc1b_bc = bcast_const(N, d["fc1_b"].ap(), "f1b")

        # ---------------- helpers ----------------
        def allgather(src_sb, rows, cols, tag):
            ag_in = dram.tile([rows, cols], BF16, tag=f"agi_{tag}")
            ag_out = dram.tile([C * rows, cols], BF16, tag=f"ago_{tag}",
                               addr_space="Shared")
            nc.gpsimd.dma_start(ag_in[:], src_sb)
            nc.gpsimd.collective_compute(
                "AllGather", OP.bypass, replica_groups=RG,
                ins=[ag_in.opt()], outs=[ag_out.opt()])
            return ag_out

        def dma_T(src_bf_2d, dst_3d, eng):
            """[128, k*128] -> [128, k, 128] chunked transpose via DMA XBAR."""
            eng.dma_start_transpose(dst_3d, src_bf_2d)

        def transpose8(src_bf, nch, dst_sb):
            """Transpose [128, nch*128] bf16 -> psum -> one DVE copy to dst."""
            tp = ps_tr.tile([P, N], BF16, tag="tr")
            for j in range(nch):
                nc.tensor.transpose(tp[:, j * P:(j + 1) * P],
                                    src_bf[:, j * P:(j + 1) * P], ident[:])
            nc.vector.tensor_copy(dst_sb, tp[:, :nch * P])

        def vb_broadcast(row_src_3d):
            """[1, C, 128] DRAM view -> [1,N] sbuf -> [128,N]."""
            vrow = sm.tile([1, N], BF16, tag="vrow", bufs=1)
            nc.sync.dma_start(vrow[:1].rearrange("o (c p) -> o c p", p=P),
                              row_src_3d)
            vb = att_p.tile([P, N], BF16, tag="vb", bufs=2)
            nc.gpsimd.partition_broadcast(vb[:], vrow[:1, :])
            return vb

        def col_extract_vb(full_3d_col, tag):
            """[128, C] column view of a gathered payload -> vb [128, N].

            transpose (PE, tiny) -> sbuf -> DRAM bounce -> broadcast."""
            tp = ps_tr.tile([P, N], BF16, tag="tr")
            nc.tensor.transpose(tp[:C, :P], full_3d_col, ident[:])
            v_sb = sm.tile([C, P], BF16, tag="vx", bufs=2)
            nc.vector.tensor_copy(v_sb[:], tp[:C, :P])
            v_dr = dram.tile([C, P], BF16, tag=f"vxd_{tag}")
            nc.sync.dma_start(v_dr[:], v_sb[:])
            return vb_broadcast(v_dr[:][None])

        def softmax_rows(u_ap, vb_ap, tagid):
            """n_bf, rs = exp(lrelu(u + v^T) masked), 1/rowsum."""
            e_bf = sc_bf.tile([P, N], BF16, tag="ebf")
            nc.vector.scalar_tensor_tensor(e_bf[:], vb_ap, u_ap, madj[:],
                                           op0=OP.add, op1=OP.add)
            nc.vector.scalar_tensor_tensor(e_bf[:], e_bf[:], GSLOPE, e_bf[:],
                                           op0=OP.mult, op1=OP.max)
            n_bf = att_p.tile([P, N], BF16, tag="nbf")
            ssum = sm.tile([P, 1], F32, tag=f"ss_{tagid}")
            nc.scalar.activation(n_bf[:], e_bf[:], AF.Exp, accum_out=ssum[:])
            rs = sm.tile([P, 1], F32, tag=f"rs_{tagid}")
            nc.vector.reciprocal(rs[:], ssum[:])
            return n_bf, rs

        def elu_store(o_ps, dst_bf, L, rs=None):
            """dst = elu(rs * o_ps); rs=None means already scaled."""
            m32 = sc_32.tile([P, N], F32, tag="s32")
            r32 = sc_32.tile([P, N], F32, tag="s32c", bufs=1)
            if rs is not None:
                nc.vector.tensor_scalar(m32[:, :L], o_ps, rs[:], 0.0,
                                        op0=OP.mult, op1=OP.min)
                nc.vector.tensor_scalar(r32[:, :L], o_ps, rs[:], 0.0,
                                        op0=OP.mult, op1=OP.max)
            else:
                nc.vector.tensor_scalar(m32[:, :L], o_ps, 0.0, None, op0=OP.min)
                nc.vector.tensor_scalar(r32[:, :L], o_ps, 0.0, None, op0=OP.max)
            g32 = sc_32.tile([P, N], F32, tag="s32b", bufs=1)
            nc.scalar.activation(g32[:, :L], m32[:, :L], AF.Exp)
            nc.vector.scalar_tensor_tensor(dst_bf, g32[:, :L], -1.0, r32[:, :L],
                                           op0=OP.add, op1=OP.add)

        # persistent per-rep tensors
        hcatT = per.tile([P, H1 * C, P], BF16, tag="hcatT")   # [128, 40, 128]
        h0f = per.tile([P, HID], F32, tag="h0f")
        uv1_sb = per.tile([P, 2 * H1], F32, tag="uv1")
        h0_full = per.tile([P, C, HID], BF16, tag="h0full")

        wg_view = [d["Wg1"].ap()[h].rearrange("(c p) f -> p c f", p=P)
                   for h in range(H1)]
        wo1_view = d["Wo1"].ap().rearrange("(g c p) f -> p g c f", p=P, c=C)
        wg2_view = [d["Wg2"].ap()[h].rearrange("(c p) f -> p c f", p=P)
                    for h in range(H2)]
        wo2_view = d["Wo2"].ap().rearrange("(c p) f -> p c f", p=P)
        fc1_view = d["fc1_w"].ap().rearrange("(c p) f -> p c f", p=P)

        for _rep in range(reps):
            # ======== GCNII h0 = lrelu(x@fc0 + b) ========
            h0_ps = ps_wh.tile([P, HID], F32, tag="wh")
            for c in range(C):
                nc.tensor.matmul(h0_ps[:], xT_sb[:, c, :], fc0_sb[:, c, :],
                                 start=(c == 0), stop=(c == C - 1))
            nc.vector.scalar_tensor_tensor(h0f[:], h0_ps[:], 1.0, fc0b_bc[:],
                                           op0=OP.mult, op1=OP.add)
            nc.vector.scalar_tensor_tensor(h0f[:], h0f[:], SLOPE, h0f[:],
                                           op0=OP.mult, op1=OP.max)
            h0b = sc_bf.tile([P, HID], BF16, tag="h0b")
            nc.vector.tensor_copy(h0b[:], h0f[:])
            dtap("d_h0f", h0f[:])

            # ======== GAT1 u,v for all heads: uv = x @ Wa1 ========
            uv1_ps = ps_uv.tile([P, 2 * H1], F32, tag="uv")
            for c in range(C):
                nc.tensor.matmul(uv1_ps[:], xT_sb[:, c, :], wa1_sb[:, c, :],
                                 start=(c == 0), stop=(c == C - 1))
            nc.vector.tensor_copy(uv1_sb[:], uv1_ps[:])
            dtap("d_uv1", uv1_sb[:])
            # v rows (cols H1..2H1) -> [5,128] for the tiny AG
            v1_bf = sc_bf.tile([P, 2 * H1], BF16, tag="v1bf")
            nc.vector.tensor_copy(v1_bf[:], uv1_sb[:])
            vtr_ps = ps_tr.tile([P, N], BF16, tag="tr")
            nc.tensor.transpose(vtr_ps[:2 * H1, :P], v1_bf[:], ident[:])
            vtr_sb = sm.tile([2 * H1, P], BF16, tag="vtr", bufs=1)
            nc.vector.tensor_copy(vtr_sb[:], vtr_ps[:2 * H1, :P])
            ag_v1 = allgather(vtr_sb[H1:2 * H1, :], H1, P, "v1")
            v1_rows = ag_v1[:].rearrange("(c h) p -> h c p", h=H1)

            # GAT1 weight stream: heads 0,1 prefetch now
            wg_sb = []
            for h in range(2):
                t = w_str.tile([P, C, N], BF16, tag="wstream")
                nc.scalar.dma_start(t[:], wg_view[h])
                wg_sb.append(t)
            wo1_sb = []
            t = w_str.tile([P, C, NC1], BF16, tag="wo1stream")
            nc.scalar.dma_start(t[:], wo1_view[:, 0])
            wo1_sb.append(t)

            wh_ps = ps_wh.tile([P, NC1], F32, tag="wh")   # o1 Wh accumulator

            # ======== GAT1: 5 heads, software-pipelined ========
            vbs = {0: vb_broadcast(v1_rows[0][None]),
                   1: vb_broadcast(v1_rows[1][None])}
            sm_state = {0: softmax_rows(uv1_sb[:, 0:1], vbs.pop(0)[:], "g1")}  # noqa
            dtap("d_n0", sm_state[0][0][:])
            o_prev = None
            for h in range(H1):
                n_bf, rs = sm_state.pop(h)
                attT = att_p.tile([P, C, P], BF16, tag="attT")
                transpose8(n_bf[:], C, attT[:].rearrange("p c m -> p (c m)"))
                if o_prev is not None:
                    transpose8(o_prev[:], C, hcatT[:, (h - 1) * C:h * C, :]
                               .rearrange("p c m -> p (c m)"))
                # z = att @ x_full
                z_ps = ps_big.tile([P, N], F32, tag="big")
                for j in range(C):
                    for s in range(2):
                        nc.tensor.matmul(z_ps[:, s * 512:(s + 1) * 512],
                                         attT[:, j, :], x_row[:, j, s * 512:(s + 1) * 512],
                                         start=(j == 0), stop=(j == C - 1))
                z_bf = sc_bf.tile([P, N], BF16, tag="zbf")
                nc.vector.tensor_scalar(z_bf[:], z_ps[:], rs[:], None, op0=OP.mult)
                if h == 0:
                    dtap("d_z0", z_bf[:])
                # o1-Wh accumulation chunks of the PREVIOUS head (fills the
                # z->zT latency window on the PE)
                if h > 0:
                    for j in range(C):
                        nc.tensor.matmul(wh_ps[:], hcatT[:, (h - 1) * C + j, :],
                                         wo1_sb[h - 1][:, j, :],
                                         start=(h == 1 and j == 0), stop=False,
                                         skip_group_check=True)
                # next head's softmax (overlaps PE work)
                if h + 2 < H1:
                    vbs[h + 2] = vb_broadcast(v1_rows[h + 2][None])
                if h + 1 < H1:
                    sm_state[h + 1] = softmax_rows(uv1_sb[:, h + 1:h + 2],
                                                   vbs.pop(h + 1)[:], "g1")
                zT = att_p.tile([P, C, P], BF16, tag="zT")
                transpose8(z_bf[:], C, zT[:].rearrange("p c m -> p (c m)"))
                # out = z @ Wg1[h]
                o_ps = ps_big.tile([P, N], F32, tag="big")
                wgh = wg_sb[h]
                for j in range(C):
                    for s in range(2):
                        nc.tensor.matmul(o_ps[:, s * 512:(s + 1) * 512],
                                         zT[:, j, :], wgh[:, j, s * 512:(s + 1) * 512],
                                         start=(j == 0), stop=(j == C - 1))
                # weight prefetches
                if h + 2 < H1:
                    t = w_str.tile([P, C, N], BF16, tag="wstream")
                    nc.scalar.dma_start(t[:], wg_view[h + 2])
                    wg_sb.append(t)
                if h + 1 < H1:
                    t = w_str.tile([P, C, NC1], BF16, tag="wo1stream")
                    nc.scalar.dma_start(t[:], wo1_view[:, h + 1])
                    wo1_sb.append(t)
                o_bf = sc_bf.tile([P, N], BF16, tag="obf")
                elu_store(o_ps[:], o_bf[:], N)
                if h == 0:
                    dtap("d_o0", o_bf[:])
                o_prev = o_bf
            # last head's outT + o1-Wh chunks
            transpose8(o_prev[:], C, hcatT[:, (H1 - 1) * C:H1 * C, :]
                       .rearrange("p c m -> p (c m)"))
            for j in range(C):
                nc.tensor.matmul(wh_ps[:], hcatT[:, (H1 - 1) * C + j, :],
                                 wo1_sb[H1 - 1][:, j, :],
                                 start=False, stop=(j == C - 1),
                                 skip_group_check=True)

            # h0 allgather (result needed only in GCNII tail)
            ag_h0 = allgather(h0b[:], P, HID, "h0")
            nc.gpsimd.dma_start(h0_full[:], ag_h0[:].rearrange("(c p) f -> p c f", p=P))

            # ======== GAT1 out-attention (o1) ========
            junk = sc_bf.tile([P, N], BF16, tag="zbf")
            uvo1 = sm.tile([P, 2], F32, tag="uvo1")
            nc.vector.scalar_tensor_tensor(junk[:, :NC1], wh_ps[:], 1.0,
                                           ao1_bc[:, :NC1], op0=OP.mult,
                                           op1=OP.mult, accum_out=uvo1[:, 0:1])
            nc.vector.scalar_tensor_tensor(junk[:, :NC1], wh_ps[:], 1.0,
                                           ao1_bc[:, NC1:], op0=OP.mult,
                                           op1=OP.mult, accum_out=uvo1[:, 1:2])
            dtap("d_uvo1", uvo1[:])
            # payload [Wh | v | pad]
            pay_wh = sc_bf.tile([P, NC1 + 8], BF16, tag="pay520")
            nc.vector.tensor_copy(pay_wh[:, :NC1], wh_ps[:])
            nc.vector.tensor_copy(pay_wh[:, NC1:NC1 + 1], uvo1[:, 1:2])
            nc.vector.memset(pay_wh[:, NC1 + 1:], 0.0)
            dtap("d_wh", pay_wh[:, :NC1])
            ag_wh = allgather(pay_wh[:], P, NC1 + 8, "wh")
            wh_full = full_p.tile([P, C, NC1 + 8], BF16, tag="full520")
            nc.gpsimd.dma_start(wh_full[:], ag_wh[:].rearrange("(c p) f -> p c f", p=P))
            vb = col_extract_vb(wh_full[:, :, NC1], "o1")
            n_bf, rs = softmax_rows(uvo1[:, 0:1], vb[:], "o1")
            attT = att_p.tile([P, C, P], BF16, tag="attT")
            transpose8(n_bf[:], C, attT[:].rearrange("p c m -> p (c m)"))
            xg_ps = ps_wh.tile([P, NC1], F32, tag="wh")
            for j in range(C):
                nc.tensor.matmul(xg_ps[:], attT[:, j, :], wh_full[:, j, :NC1],
                                 start=(j == 0), stop=(j == C - 1))
            # xg = elu(rs * xg_ps) -> payload [xg | v1 v2 | pad]
            pay_g2 = sc_bf.tile([P, NC1 + 8], BF16, tag="pay520")
            elu_store(xg_ps[:], pay_g2[:, :NC1], NC1, rs=rs)
            xgT = att_p.tile([P, 4, P], BF16, tag="xgT")
            transpose8(pay_g2[:, :NC1], 4, xgT[:].rearrange("p c m -> p (c m)"))
            uv2_ps = ps_uv.tile([P, 2 * H2], F32, tag="uv")
            for c in range(4):
                nc.tensor.matmul(uv2_ps[:], xgT[:, c, :], wa2_sb[:, c, :],
                                 start=(c == 0), stop=(c == 3))
            uv2_sb = sm.tile([P, 2 * H2], F32, tag="uv2")
            nc.vector.tensor_copy(uv2_sb[:], uv2_ps[:])
            nc.vector.tensor_copy(pay_g2[:, NC1:NC1 + 2], uv2_sb[:, H2:])
            nc.vector.memset(pay_g2[:, NC1 + 2:], 0.0)
            dtap("d_xg", pay_g2[:])
            dtap("d_uv2", uv2_sb[:])
            ag_xg = allgather(pay_g2[:], P, NC1 + 8, "xg")
            xg_full = full_p.tile([P, C, NC1 + 8], BF16, tag="full520")
            nc.gpsimd.dma_start(xg_full[:], ag_xg[:].rearrange("(c p) f -> p c f", p=P))
            # wg2 stream (needed from here on)
            wg2_sb = w_str.tile([P, H2, 4, NC1], BF16, tag="wo1stream")
            for h in range(H2):
                nc.scalar.dma_start(wg2_sb[:, h], wg2_view[h])

            # ======== GAT2: 2 heads (pipelined) ========
            pay_o2 = sc_bf.tile([P, N + 16], BF16, tag="payo2", bufs=1)
            vbs2 = {h: col_extract_vb(xg_full[:, :, NC1 + h], f"g2_{h}")
                    for h in range(H2)}
            sm2 = {0: softmax_rows(uv2_sb[:, 0:1], vbs2.pop(0)[:], "g2")}
            for h in range(H2):
                n_bf, rs = sm2.pop(h)
                if h == 0:
                    dtap("d_n20", n_bf[:])
                attT = att_p.tile([P, C, P], BF16, tag="attT")
                transpose8(n_bf[:], C, attT[:].rearrange("p c m -> p (c m)"))
                z_ps = ps_wh.tile([P, NC1], F32, tag="wh")
                for j in range(C):
                    nc.tensor.matmul(z_ps[:], attT[:, j, :],
                                     xg_full[:, j, :NC1],
                                     start=(j == 0), stop=(j == C - 1))
                z_bf = sc_bf.tile([P, NC1], BF16, tag="h0b")
                nc.vector.tensor_scalar(z_bf[:], z_ps[:], rs[:], None, op0=OP.mult)
                if h == 0:
                    dtap("d_z20", z_bf[:])
                zT = att_p.tile([P, 4, P], BF16, tag="xgT")
                transpose8(z_bf[:], 4, zT[:].rearrange("p c m -> p (c m)"))
                if h + 1 < H2:
                    sm2[h + 1] = softmax_rows(uv2_sb[:, h + 1:h + 2],
                                              vbs2.pop(h + 1)[:], "g2")
                o_ps = ps_wh.tile([P, NC1], F32, tag="wh")
                for j in range(4):
                    nc.tensor.matmul(o_ps[:], zT[:, j, :], wg2_sb[:, h, j, :],
                                     start=(j == 0), stop=(j == 3))
                elu_store(o_ps[:], pay_o2[:, h * NC1:(h + 1) * NC1], NC1)

            # ======== GAT2 out-attention (o2) ========
            junk2 = sc_bf.tile([P, N], BF16, tag="zbf")
            uvo2 = sm.tile([P, 2], F32, tag="uvo2")
            nc.vector.scalar_tensor_tensor(junk2[:], pay_o2[:, :N], 1.0,
                                           woa2u_bc[:], op0=OP.mult,
                                           op1=OP.mult, accum_out=uvo2[:, 0:1])
            nc.vector.scalar_tensor_tensor(junk2[:], pay_o2[:, :N], 1.0,
                                           woa2v_bc[:], op0=OP.mult,
                                           op1=OP.mult, accum_out=uvo2[:, 1:2])
            nc.vector.tensor_copy(pay_o2[:, N:N + 1], uvo2[:, 1:2])
            nc.vector.memset(pay_o2[:, N + 1:], 0.0)
            dtap("d_hc2", pay_o2[:])
            ag_h2 = allgather(pay_o2[:], P, N + 16, "h2")
            h2_full = full_p.tile([P, C, N + 16], BF16, tag="h2full", bufs=1)
            nc.gpsimd.dma_start(h2_full[:], ag_h2[:].rearrange("(c p) f -> p c f", p=P))
            # Wo2 stream (during the AG)
            wo2_sb = w_str.tile([P, C, N], BF16, tag="wstream")
            nc.scalar.dma_start(wo2_sb[:], wo2_view)
            vb = col_extract_vb(h2_full[:, :, N], "o2")
            n_bf, rs = softmax_rows(uvo2[:, 0:1], vb[:], "o2")
            attT = att_p.tile([P, C, P], BF16, tag="attT")
            transpose8(n_bf[:], C, attT[:].rearrange("p c m -> p (c m)"))
            # z = att @ hcat2_full
            z_ps = ps_big.tile([P, N], F32, tag="big")
            for j in range(C):
                for s in range(2):
                    nc.tensor.matmul(z_ps[:, s * 512:(s + 1) * 512],
                                     attT[:, j, :],
                                     h2_full[:, j, s * 512:(s + 1) * 512],
                                     start=(j == 0), stop=(j == C - 1))
            z_bf = sc_bf.tile([P, N], BF16, tag="zbf")
            nc.vector.tensor_scalar(z_bf[:], z_ps[:], rs[:], None, op0=OP.mult)
            zT = att_p.tile([P, C, P], BF16, tag="zT")
            transpose8(z_bf[:], C, zT[:].rearrange("p c m -> p (c m)"))
            o_ps = ps_big.tile([P, N], F32, tag="big")
            for j in range(C):
                for s in range(2):
                    nc.tensor.matmul(o_ps[:, s * 512:(s + 1) * 512],
                                     zT[:, j, :], wo2_sb[:, j, s * 512:(s + 1) * 512],
                                     start=(j == 0), stop=(j == C - 1))
            xg2_bf = sc_bf.tile([P, N], BF16, tag="obf")
            elu_store(o_ps[:], xg2_bf[:], N)
            dtap("d_xg2", xg2_bf[:])
            xg2T = att_p.tile([P, C, P], BF16, tag="zT")
            transpose8(xg2_bf[:], C, xg2T[:].rearrange("p c m -> p (c m)"))

            # ======== GCNII ========
            hi_ps = ps_wh.tile([P, HID], F32, tag="wh")
            for j in range(C):
                nc.tensor.matmul(hi_ps[:], xg2T[:, j, :], h0_full[:, j, :],
                                 start=(j == 0), stop=(j == C - 1))
            sf = sc_32.tile([P, HID], F32, tag="sf", bufs=1)
            nc.vector.scalar_tensor_tensor(sf[:], hi_ps[:], 9.0, h0f[:],
                                           op0=OP.mult, op1=OP.add)
            nc.vector.tensor_scalar(sf[:], sf[:], 0.1, None, op0=OP.mult)
            s_bf = sc_bf.tile([P, HID], BF16, tag="h0b")
            nc.vector.tensor_copy(s_bf[:], sf[:])
            dtap("d_sf", sf[:])
            ag_s = allgather(s_bf[:], P, HID, "s")
            s_full = full_p.tile([P, C, HID], BF16, tag="sfull", bufs=1)
            nc.gpsimd.dma_start(s_full[:], ag_s[:].rearrange("(c p) f -> p c f", p=P))
            fc1_sb = w_str.tile([P, 4, N], BF16, tag="wo1stream")
            nc.scalar.dma_start(fc1_sb[:], fc1_view)
            mm_ps = ps_wh.tile([P, HID], F32, tag="wh")
            for c in range(C):
                nc.tensor.matmul(mm_ps[:], cw1T_sb[:, c, :], s_full[:, c, :],
                                 start=(c == 0), stop=(c == C - 1))
            hf = sc_32.tile([P, HID], F32, tag="s32")
            nc.vector.scalar_tensor_tensor(hf[:], sf[:], (1.0 - THETA2) / THETA2,
                                           mm_ps[:], op0=OP.mult, op1=OP.add)
            nc.vector.scalar_tensor_tensor(hf[:], hf[:], THETA2, h0f[:],
                                           op0=OP.mult, op1=OP.add)
            nc.vector.scalar_tensor_tensor(hf[:], hf[:], SLOPE, hf[:],
                                           op0=OP.mult, op1=OP.max)
            hb = sc_bf.tile([P, HID], BF16, tag="h0b")
            nc.vector.tensor_copy(hb[:], hf[:])
            hT = att_p.tile([P, 4, P], BF16, tag="xgT")
            transpose8(hb[:], 4, hT[:].rearrange("p c m -> p (c m)"))
            y_ps = ps_big.tile([P, N], F32, tag="big")
            for c in range(4):
                for s in range(2):
                    nc.tensor.matmul(y_ps[:, s * 512:(s + 1) * 512], hT[:, c, :],
                                     fc1_sb[:, c, s * 512:(s + 1) * 512],
                                     start=(c == 0), stop=(c == 3))
            y_sb = sc_32.tile([P, N], F32, tag="s32")
            nc.vector.scalar_tensor_tensor(y_sb[:], y_ps[:], 1.0, fc1b_bc[:],
                                           op0=OP.mult, op1=OP.add)
            nc.sync.dma_start(out_d.ap(), y_sb[:])


def _shard_inputs(inputs):
    f32 = lambda a: np.asarray(a, dtype=np.float32)
    bf = lambda a: np.ascontiguousarray(f32(a)).astype(ml_dtypes.bfloat16)
    x = f32(inputs["x"])
    adj = f32(inputs["adj"])
    x_bf = bf(x)
    xT_bf = np.ascontiguousarray(x_bf.T)
    cw1T = np.ascontiguousarray(bf(inputs["cw1"]).T)
    Wg1 = f32(inputs["Wg1"])
    ag1 = f32(inputs["ag1"])[:, :, 0]          # [5, 2048]
    Wa1 = np.stack([Wg1[h] @ ag1[h, :N] for h in range(H1)] +
                   [Wg1[h] @ ag1[h, N:] for h in range(H1)], axis=1)  # [1024, 10]
    Wg2 = f32(inputs["Wg2"])
    ag2 = f32(inputs["ag2"])[:, :, 0]          # [2, 1024]
    Wa2 = np.stack([Wg2[h] @ ag2[h, :NC1] for h in range(H2)] +
                   [Wg2[h] @ ag2[h, NC1:] for h in range(H2)], axis=1)  # [512, 4]
    Wo2 = f32(inputs["Wo2"])
    ao2 = f32(inputs["ao2"])[:, 0]             # [2048]
    Woa2 = np.stack([Wo2 @ ao2[:N], Wo2 @ ao2[N:]], axis=1)  # [1024, 2]
    shared = {
        "x_row": x_bf,
        "Wg1": bf(Wg1),
        "Wa1": bf(Wa1),
        "Wo1": bf(inputs["Wo1"]),
        "ao1": bf(f32(inputs["ao1"])[:, 0]),
        "Wg2": bf(Wg2),
        "Wa2": bf(Wa2),
        "Wo2": bf(Wo2),
        "Woa2": bf(Woa2),
        "fc0_w": bf(inputs["fc0_w"]),
        "fc0_b": bf(inputs["fc0_b"]),
        "fc1_w": bf(inputs["fc1_w"]),
        "fc1_b": bf(inputs["fc1_b"]),
    }
    in_maps = []
    for c in range(C):
        r0, r1 = c * P, (c + 1) * P
        m = dict(shared)
        m["xT_sl"] = np.ascontiguousarray(xT_bf[:, r0:r1])
        m["adj_r"] = np.ascontiguousarray(adj[r0:r1])
        m["cw1T_sl"] = np.ascontiguousarray(cw1T[:, r0:r1])
        in_maps.append(m)
    return in_maps


def kernel(**inputs) -> np.ndarray:
    if "nc" not in _CACHE:
        _CACHE["nc"] = _build()
    nc = _CACHE["nc"]
    in_maps = _shard_inputs(inputs)
    res = run_bass_kernel_spmd(nc, in_maps, core_ids=list(range(C)))
    out = np.concatenate([res.results[c]["out"] for c in range(C)], axis=0)
    return np.asarray(out, dtype=np.float32)


# revision 22
# speedup vs baseline: 1.4163x; 1.0489x over previous
"""Trainium2 Bass kernel for nn_GCNII_80178449482260 (2x dense GAT + GCNII).

Row-parallel over N=1024 across 8 cores (128 rows each), restructured to
minimize collectives and keep the PE continuously busy (p-state ramp):

  * GAT layers use  att @ Wh == (att @ h_in) @ W  and
    u,v == h_in @ (W @ a_half)  with W@a precomputed on host.  Since x is
    replicated, the five GAT1 heads need NO big all-gather -- just one tiny
    [5,128] v-gather.  GAT2's two heads share one input all-gather.
  * Collectives (6): v1(tiny), h0, [Wh_o1|v], [xg|v], [hcat2|v], support.
    All outputs in Shared DRAM; v vectors ride inside the payloads.
  * All [128, k*128] transposes via DMA XBAR (dma_start_transpose) on the
    two HWDGE rings (sync + scalar) -- zero PE/DVE cost.
  * Scalar engine runs ONLY Exp; lrelu/elu masks on DVE (bf16 where safe);
    PSUM->SBUF copies on DVE (gpsimd has no PSUM port).
  * Weights stream as single big DMAs (split across 16 SDMA engines).
  * Per-head software pipelining: head h+1's softmax chain and the o1-Wh
    accumulation of head h-1 are interleaved into head h's matmul stream.
"""
import os
import sys
import numpy as np

for _p in ("/opt/trn_rl_repo", "/root/.axon_site/_ro/trn_rl_repo"):
    if _p not in sys.path:
        sys.path.insert(0, _p)

import ml_dtypes  # noqa: E402
from concourse import bacc, tile, mybir  # noqa: E402
from concourse.bass_utils import run_bass_kernel_spmd  # noqa: E402
from concourse.kernels.tile_matmul import make_identity  # noqa: E402

BF16 = mybir.dt.bfloat16
F32 = mybir.dt.float32
AF = mybir.ActivationFunctionType
OP = mybir.AluOpType

N = 1024      # nodes
P = 128       # partitions / rows per core
C = 8         # cores
HID = 512
NC1 = 512
H1, H2 = 5, 2
THETA2 = float(np.log(1.25))   # GCNII layer-2 theta; layer 1 is dead code
SLOPE = 0.25
GSLOPE = 0.01                  # GAT leaky-relu slope
RG = [list(range(C))]

_CACHE = {}


def _build(reps=1):
    nc = bacc.Bacc("TRN2", target_bir_lowering=False, debug=False,
                   num_devices=C)
    d = {}
    d["xT_sl"] = nc.dram_tensor("xT_sl", [N, P], BF16, kind="ExternalInput")
    d["x_row"] = nc.dram_tensor("x_row", [N, N], BF16, kind="ExternalInput")
    d["adj_r"] = nc.dram_tensor("adj_r", [P, N], F32, kind="ExternalInput")
    d["Wg1"] = nc.dram_tensor("Wg1", [H1, N, N], BF16, kind="ExternalInput")
    d["Wa1"] = nc.dram_tensor("Wa1", [N, 2 * H1], BF16, kind="ExternalInput")
    d["Wo1"] = nc.dram_tensor("Wo1", [H1 * N, NC1], BF16, kind="ExternalInput")
    d["ao1"] = nc.dram_tensor("ao1", [2 * NC1], BF16, kind="ExternalInput")
    d["Wg2"] = nc.dram_tensor("Wg2", [H2, NC1, NC1], BF16, kind="ExternalInput")
    d["Wa2"] = nc.dram_tensor("Wa2", [NC1, 2 * H2], BF16, kind="ExternalInput")
    d["Wo2"] = nc.dram_tensor("Wo2", [N, N], BF16, kind="ExternalInput")
    d["Woa2"] = nc.dram_tensor("Woa2", [N, 2], BF16, kind="ExternalInput")
    d["fc0_w"] = nc.dram_tensor("fc0_w", [N, HID], BF16, kind="ExternalInput")
    d["fc0_b"] = nc.dram_tensor("fc0_b", [HID], BF16, kind="ExternalInput")
    d["fc1_w"] = nc.dram_tensor("fc1_w", [HID, N], BF16, kind="ExternalInput")
    d["fc1_b"] = nc.dram_tensor("fc1_b", [N], BF16, kind="ExternalInput")
    d["cw1T_sl"] = nc.dram_tensor("cw1T_sl", [N, P], BF16, kind="ExternalInput")
    out_d = nc.dram_tensor("out", [P, N], F32, kind="ExternalOutput")
    dbg = {}
    if os.environ.get("KDEBUG"):
        for nm, shp, dt in [("d_h0f", [P, HID], F32), ("d_uv1", [P, 2 * H1], F32),
                            ("d_n0", [P, N], BF16),
                            ("d_z0", [P, N], BF16), ("d_o0", [P, N], BF16),
                            ("d_wh", [P, NC1], BF16), ("d_uvo1", [P, 2], F32),
                            ("d_xg", [P, NC1 + 8], BF16),
                            ("d_uv2", [P, 2 * H2], F32),
                            ("d_hc2", [P, N + 16], BF16),
                            ("d_n20", [P, N], BF16), ("d_z20", [P, NC1], BF16),
                            ("d_xg2", [P, N], BF16), ("d_sf", [P, HID], F32)]:
            dbg[nm] = nc.dram_tensor(nm, shp, dt, kind="ExternalOutput")

    with tile.TileContext(nc) as tc:
        _body(nc, tc, d, out_d, reps, dbg)
    nc.compile()
    return nc


def _body(nc, tc, d, out_d, reps=1, dbg=None):
    dbg = dbg or {}

    def dtap(nm, ap):
        if nm in dbg:
            nc.sync.dma_start(dbg[nm].ap(), ap)

    with (
        tc.tile_pool(name="cst", bufs=1) as cst,          # constants
        tc.tile_pool(name="per", bufs=1) as per,          # cross-phase persistents
        tc.tile_pool(name="wstr", bufs=2) as w_str,       # big weight stream
        tc.tile_pool(name="full", bufs=2) as full_p,      # gathered full tensors
        tc.tile_pool(name="att", bufs=2) as att_p,        # per-layer attention
        tc.tile_pool(name="scbf", bufs=2) as sc_bf,       # bf16 scratch
        tc.tile_pool(name="sc32", bufs=2) as sc_32,       # f32 scratch
        tc.tile_pool(name="sm", bufs=2) as sm,            # small vectors
        tc.tile_pool(name="psbig", bufs=2, space="PSUM") as ps_big,   # [128,1024] f32
        tc.tile_pool(name="pswh", bufs=1, space="PSUM") as ps_wh,     # [128,512] f32
        tc.tile_pool(name="pstr", bufs=2, space="PSUM") as ps_tr,     # [128,1024] bf16
        tc.tile_pool(name="psuv", bufs=1, space="PSUM") as ps_uv,     # tiny f32
        tc.tile_pool(name="dram", bufs=1, space="DRAM") as dram,
    ):
        ident = cst.tile([P, P], BF16, tag="ident")
        make_identity(nc, ident)

        # ---------- constant / persistent loads ----------
        xT_sb = per.tile([P, C, P], BF16, tag="xT")
        nc.scalar.dma_start(xT_sb[:], d["xT_sl"].ap().rearrange("(c p) m -> p c m", p=P))
        x_row = per.tile([P, C, N], BF16, tag="x_row")
        nc.scalar.dma_start(x_row[:], d["x_row"].ap().rearrange("(c p) f -> p c f", p=P))
        wa1_sb = cst.tile([P, C, 2 * H1], BF16, tag="wa1")
        nc.scalar.dma_start(wa1_sb[:], d["Wa1"].ap().rearrange("(c p) f -> p c f", p=P))
        fc0_sb = cst.tile([P, C, HID], BF16, tag="fc0")
        nc.scalar.dma_start(fc0_sb[:], d["fc0_w"].ap().rearrange("(c p) f -> p c f", p=P))
        wa2_sb = cst.tile([P, 4, 2 * H2], BF16, tag="wa2")
        nc.scalar.dma_start(wa2_sb[:], d["Wa2"].ap().rearrange("(c p) f -> p c f", p=P))
        cw1T_sb = cst.tile([P, C, P], BF16, tag="cw1T")
        nc.scalar.dma_start(cw1T_sb[:], d["cw1T_sl"].ap().rearrange("(c p) m -> p c m", p=P))

        adj_sb = sc_32.tile([P, N], F32, tag="s32")
        nc.scalar.dma_start(adj_sb[:], d["adj_r"].ap())
        madj = per.tile([P, N], BF16, tag="madj")        # 0 where adj>0 else -9e15
        nc.vector.tensor_scalar(madj[:], adj_sb[:], 0.0, None, op0=OP.is_gt)
        nc.vector.tensor_scalar(madj[:], madj[:], 1.0, 9e15,
                                op0=OP.subtract, op1=OP.mult)

        def bcast_const(L, src_ap, tag):
            row = sm.tile([1, N], BF16, tag="vrow", bufs=1)
            nc.gpsimd.dma_start(row[:1, :L], src_ap[None, :])
            bc = cst.tile([P, L], BF16, tag=f"bc_{tag}")
            nc.gpsimd.partition_broadcast(bc[:], row[:1, :L])
            return bc

        ao1_bc = bcast_const(2 * NC1, d["ao1"].ap(), "ao1")
        woa2u_bc = bcast_const(N, d["Woa2"].ap()[:, 0], "w2u")
        woa2v_bc = bcast_const(N, d["Woa2"].ap()[:, 1], "w2v")
        fc0b_bc = bcast_const(HID, d["fc0_b"].ap(), "f0b")
        fc1b_bc = bcast_const(N, d["fc1_b"].ap(), "f1b")

        # ---------------- helpers ----------------
        def allgather(src_sb, rows, cols, tag):
            ag_in = dram.tile([rows, cols], BF16, tag=f"agi_{tag}")
            ag_out = dram.tile([C * rows, cols], BF16, tag=f"ago_{tag}",
                               addr_space="Shared")
            nc.gpsimd.dma_start(ag_in[:], src_sb)
            nc.gpsimd.collective_compute(
                "AllGather", OP.bypass, replica_groups=RG,
                ins=[ag_in.opt()], outs=[ag_out.opt()])
            return ag_out

        def dma_T(src_bf_2d, dst_3d, eng):
            """[128, k*128] -> [128, k, 128] chunked transpose via DMA XBAR."""
            eng.dma_start_transpose(dst_3d, src_bf_2d)

        def transpose8(src_bf, nch, dst_sb):
            """Transpose [128, nch*128] bf16 -> psum -> one DVE copy to dst."""
            tp = ps_tr.tile([P, N], BF16, tag="tr")
            for j in range(nch):
                nc.tensor.transpose(tp[:, j * P:(j + 1) * P],
                                    src_bf[:, j * P:(j + 1) * P], ident[:])
            nc.vector.tensor_copy(dst_sb, tp[:, :nch * P])

        def vb_broadcast(row_src_3d):
            """[1, C, 128] DRAM view -> [1,N] sbuf -> [128,N]."""
            vrow = sm.tile([1, N], BF16, tag="vrow", bufs=1)
            nc.sync.dma_start(vrow[:1].rearrange("o (c p) -> o c p", p=P),
                              row_src_3d)
            vb = att_p.tile([P, N], BF16, tag="vb", bufs=2)
            nc.gpsimd.partition_broadcast(vb[:], vrow[:1, :])
            return vb

        def col_extract_vb(full_3d_col, tag):
            """[128, C] column view of a gathered payload -> vb [128, N].

            transpose (PE, tiny) -> sbuf -> DRAM bounce -> broadcast."""
            tp = ps_tr.tile([P, N], BF16, tag="tr")
            nc.tensor.transpose(tp[:C, :P], full_3d_col, ident[:])
            v_sb = sm.tile([C, P], BF16, tag="vx", bufs=2)
            nc.vector.tensor_copy(v_sb[:], tp[:C, :P])
            v_dr = dram.tile([C, P], BF16, tag=f"vxd_{tag}")
            nc.sync.dma_start(v_dr[:], v_sb[:])
            return vb_broadcast(v_dr[:][None])

        def softmax_rows(u_ap, vb_ap, tagid):
            """n_bf, rs = exp(lrelu(u + v^T) masked), 1/rowsum."""
            e_bf = sc_bf.tile([P, N], BF16, tag="ebf")
            nc.vector.scalar_tensor_tensor(e_bf[:], vb_ap, u_ap, madj[:],
                                           op0=OP.add, op1=OP.add)
            nc.vector.scalar_tensor_tensor(e_bf[:], e_bf[:], GSLOPE, e_bf[:],
                                           op0=OP.mult, op1=OP.max)
            n_bf = att_p.tile([P, N], BF16, tag="nbf")
            ssum = sm.tile([P, 1], F32, tag=f"ss_{tagid}")
            nc.scalar.activation(n_bf[:], e_bf[:], AF.Exp, accum_out=ssum[:])
            rs = sm.tile([P, 1], F32, tag=f"rs_{tagid}")
            nc.vector.reciprocal(rs[:], ssum[:])
            return n_bf, rs

        def elu_store(o_ps, dst_bf, L, rs=None):
            """dst = elu(rs * o_ps); rs=None means already scaled."""
            m32 = sc_32.tile([P, N], F32, tag="s32")
            r32 = sc_32.tile([P, N], F32, tag="s32c", bufs=1)
            if rs is not None:
                nc.vector.tensor_scalar(m32[:, :L], o_ps, rs[:], 0.0,
                                        op0=OP.mult, op1=OP.min)
                nc.vector.tensor_scalar(r32[:, :L], o_ps, rs[:], 0.0,
                                        op0=OP.mult, op1=OP.max)
            else:
                nc.vector.tensor_scalar(m32[:, :L], o_ps, 0.0, None, op0=OP.min)
                nc.vector.tensor_scalar(r32[:, :L], o_ps, 0.0, None, op0=OP.max)
            g32 = sc_32.tile([P, N], F32, tag="s32b", bufs=1)
            nc.scalar.activation(g32[:, :L], m32[:, :L], AF.Exp)
            nc.vector.scalar_tensor_tensor(dst_bf, g32[:, :L], -1.0, r32[:, :L],
                                           op0=OP.add, op1=OP.add)

        # persistent per-rep tensors
        hcatT = per.tile([P, H1 * C, P], BF16, tag="hcatT")   # [128, 40, 128]
        h0f = per.tile([P, HID], F32, tag="h0f")
        uv1_sb = per.tile([P, 2 * H1], F32, tag="uv1")
        h0_full = per.tile([P, C, HID], BF16, tag="h0full")

        wg_view = [d["Wg1"].ap()[h].rearrange("(c p) f -> p c f", p=P)
                   for h in range(H1)]
        wo1_view = d["Wo1"].ap().rearrange("(g c p) f -> p g c f", p=P, c=C)
        wg2_view = [d["Wg2"].ap()[h].rearrange("(c p) f -> p c f", p=P)
                    for h in range(H2)]
        wo2_view = d["Wo2"].ap().rearrange("(c p) f -> p c f", p=P)
        fc1_view = d["fc1_w"].ap().rearrange("(c p) f -> p c f", p=P)

        for _rep in range(reps):
            # ======== GCNII h0 = lrelu(x@fc0 + b) ========
            h0_ps = ps_wh.tile([P, HID], F32, tag="wh")
            for c in range(C):
                nc.tensor.matmul(h0_ps[:], xT_sb[:, c, :], fc0_sb[:, c, :],
                                 start=(c == 0), stop=(c == C - 1))
            nc.vector.scalar_tensor_tensor(h0f[:], h0_ps[:], 1.0, fc0b_bc[:],
                                           op0=OP.mult, op1=OP.add)
            nc.vector.scalar_tensor_tensor(h0f[:], h0f[:], SLOPE, h0f[:],
                                           op0=OP.mult, op1=OP.max)
            h0b = sc_bf.tile([P, HID], BF16, tag="h0b")
            nc.vector.tensor_copy(h0b[:], h0f[:])
            dtap("d_h0f", h0f[:])

            # ======== GAT1 u,v for all heads: uv = x @ Wa1 ========
            uv1_ps = ps_uv.tile([P, 2 * H1], F32, tag="uv")
            for c in range(C):
                nc.tensor.matmul(uv1_ps[:], xT_sb[:, c, :], wa1_sb[:, c, :],
                                 start=(c == 0), stop=(c == C - 1))
            nc.vector.tensor_copy(uv1_sb[:], uv1_ps[:])
            dtap("d_uv1", uv1_sb[:])
            # v rows (cols H1..2H1) -> [5,128] for the tiny AG
            v1_bf = sc_bf.tile([P, 2 * H1], BF16, tag="v1bf")
            nc.vector.tensor_copy(v1_bf[:], uv1_sb[:])
            vtr_ps = ps_tr.tile([P, N], BF16, tag="tr")
            nc.tensor.transpose(vtr_ps[:2 * H1, :P], v1_bf[:], ident[:])
            vtr_sb = sm.tile([2 * H1, P], BF16, tag="vtr", bufs=1)
            nc.vector.tensor_copy(vtr_sb[:], vtr_ps[:2 * H1, :P])
            ag_v1 = allgather(vtr_sb[H1:2 * H1, :], H1, P, "v1")
            v1_rows = ag_v1[:].rearrange("(c h) p -> h c p", h=H1)

            # GAT1 weight stream: heads 0,1 prefetch now
            wg_sb = []
            for h in range(2):
                t = w_str.tile([P, C, N], BF16, tag="wstream")
                nc.sync.dma_start(t[:], wg_view[h])
                wg_sb.append(t)
            wo1_sb = []
            t = w_str.tile([P, C, NC1], BF16, tag="wo1stream")
            nc.sync.dma_start(t[:], wo1_view[:, 0])
            wo1_sb.append(t)

            wh_ps = ps_wh.tile([P, NC1], F32, tag="wh")   # o1 Wh accumulator

            # ======== GAT1: 5 heads, software-pipelined ========
            vbs = {0: vb_broadcast(v1_rows[0][None]),
                   1: vb_broadcast(v1_rows[1][None])}
            sm_state = {0: softmax_rows(uv1_sb[:, 0:1], vbs.pop(0)[:], "g1")}  # noqa
            dtap("d_n0", sm_state[0][0][:])
            sm_state[1] = softmax_rows(uv1_sb[:, 1:2], vbs.pop(1)[:], "g1")
            attTs = {0: att_p.tile([P, C, P], BF16, tag="attT", name="attTp0")}
            transpose8(sm_state[0][0][:], C,
                       attTs[0][:].rearrange("p c m -> p (c m)"))
            o_prev = None
            for h in range(H1):
                n_bf, rs = sm_state.pop(h)
                attT = attTs.pop(h)
                # z = att @ x_full
                z_ps = ps_big.tile([P, N], F32, tag="big")
                for j in range(C):
                    for s in range(2):
                        nc.tensor.matmul(z_ps[:, s * 512:(s + 1) * 512],
                                         attT[:, j, :], x_row[:, j, s * 512:(s + 1) * 512],
                                         start=(j == 0), stop=(j == C - 1))
                z_bf = sc_bf.tile([P, N], BF16, tag="zbf")
                nc.vector.tensor_scalar(z_bf[:], z_ps[:], rs[:], None, op0=OP.mult)
                if h == 0:
                    dtap("d_z0", z_bf[:])
                # outT + o1-Wh chunks of the previous head (PE filler while
                # the DVE produces z_bf)
                if o_prev is not None:
                    transpose8(o_prev[:], C, hcatT[:, (h - 1) * C:h * C, :]
                               .rearrange("p c m -> p (c m)"))
                    for j in range(C):
                        nc.tensor.matmul(wh_ps[:], hcatT[:, (h - 1) * C + j, :],
                                         wo1_sb[h - 1][:, j, :],
                                         start=(h == 1 and j == 0), stop=False,
                                         skip_group_check=True)
                # attT transposes for the NEXT head (softmax already done)
                if h + 1 < H1:
                    attTs[h + 1] = att_p.tile([P, C, P], BF16, tag="attT",
                                              name=f"attTp{h + 1}")
                    transpose8(sm_state[h + 1][0][:], C,
                               attTs[h + 1][:].rearrange("p c m -> p (c m)"))
                zT = att_p.tile([P, C, P], BF16, tag="zT")
                transpose8(z_bf[:], C, zT[:].rearrange("p c m -> p (c m)"))
                # out = z @ Wg1[h]
                o_ps = ps_big.tile([P, N], F32, tag="big")
                wgh = wg_sb[h]
                for j in range(C):
                    for s in range(2):
                        nc.tensor.matmul(o_ps[:, s * 512:(s + 1) * 512],
                                         zT[:, j, :], wgh[:, j, s * 512:(s + 1) * 512],
                                         start=(j == 0), stop=(j == C - 1))
                # prefetches + next-next softmax
                if h + 2 < H1:
                    vbs[h + 2] = vb_broadcast(v1_rows[h + 2][None])
                    t = w_str.tile([P, C, N], BF16, tag="wstream")
                    nc.sync.dma_start(t[:], wg_view[h + 2])
                    wg_sb.append(t)
                if h + 1 < H1:
                    t = w_str.tile([P, C, NC1], BF16, tag="wo1stream")
                    nc.sync.dma_start(t[:], wo1_view[:, h + 1])
                    wo1_sb.append(t)
                o_bf = sc_bf.tile([P, N], BF16, tag="obf")
                elu_store(o_ps[:], o_bf[:], N)
                if h == 0:
                    dtap("d_o0", o_bf[:])
                if h + 2 < H1:
                    sm_state[h + 2] = softmax_rows(uv1_sb[:, h + 2:h + 3],
                                                   vbs.pop(h + 2)[:], "g1")
                o_prev = o_bf
            # last head's outT + o1-Wh chunks
            transpose8(o_prev[:], C, hcatT[:, (H1 - 1) * C:H1 * C, :]
                       .rearrange("p c m -> p (c m)"))
            for j in range(C):
                nc.tensor.matmul(wh_ps[:], hcatT[:, (H1 - 1) * C + j, :],
                                 wo1_sb[H1 - 1][:, j, :],
                                 start=False, stop=(j == C - 1),
                                 skip_group_check=True)

            # h0 allgather (result needed only in GCNII tail)
            ag_h0 = allgather(h0b[:], P, HID, "h0")
            nc.gpsimd.dma_start(h0_full[:], ag_h0[:].rearrange("(c p) f -> p c f", p=P))

            # ======== GAT1 out-attention (o1) ========
            junk = sc_bf.tile([P, N], BF16, tag="zbf")
            uvo1 = sm.tile([P, 2], F32, tag="uvo1")
            nc.vector.scalar_tensor_tensor(junk[:, :NC1], wh_ps[:], 1.0,
                                           ao1_bc[:, :NC1], op0=OP.mult,
                                           op1=OP.mult, accum_out=uvo1[:, 0:1])
            nc.vector.scalar_tensor_tensor(junk[:, :NC1], wh_ps[:], 1.0,
                                           ao1_bc[:, NC1:], op0=OP.mult,
                                           op1=OP.mult, accum_out=uvo1[:, 1:2])
            dtap("d_uvo1", uvo1[:])
            # payload [Wh | v | pad]
            pay_wh = sc_bf.tile([P, NC1 + 8], BF16, tag="pay520")
            nc.vector.tensor_copy(pay_wh[:, :NC1], wh_ps[:])
            nc.vector.tensor_copy(pay_wh[:, NC1:NC1 + 1], uvo1[:, 1:2])
            nc.vector.memset(pay_wh[:, NC1 + 1:], 0.0)
            dtap("d_wh", pay_wh[:, :NC1])
            ag_wh = allgather(pay_wh[:], P, NC1 + 8, "wh")
            wh_full = full_p.tile([P, C, NC1 + 8], BF16, tag="full520")
            nc.gpsimd.dma_start(wh_full[:], ag_wh[:].rearrange("(c p) f -> p c f", p=P))
            vb = col_extract_vb(wh_full[:, :, NC1], "o1")
            n_bf, rs = softmax_rows(uvo1[:, 0:1], vb[:], "o1")
            attT = att_p.tile([P, C, P], BF16, tag="attT")
            transpose8(n_bf[:], C, attT[:].rearrange("p c m -> p (c m)"))
            xg_ps = ps_wh.tile([P, NC1], F32, tag="wh")
            for j in range(C):
                nc.tensor.matmul(xg_ps[:], attT[:, j, :], wh_full[:, j, :NC1],
                                 start=(j == 0), stop=(j == C - 1))
            # xg = elu(rs * xg_ps) -> payload [xg | v1 v2 | pad]
            pay_g2 = sc_bf.tile([P, NC1 + 8], BF16, tag="pay520")
            elu_store(xg_ps[:], pay_g2[:, :NC1], NC1, rs=rs)
            xgT = att_p.tile([P, 4, P], BF16, tag="xgT")
            transpose8(pay_g2[:, :NC1], 4, xgT[:].rearrange("p c m -> p (c m)"))
            uv2_ps = ps_uv.tile([P, 2 * H2], F32, tag="uv")
            for c in range(4):
                nc.tensor.matmul(uv2_ps[:], xgT[:, c, :], wa2_sb[:, c, :],
                                 start=(c == 0), stop=(c == 3))
            uv2_sb = sm.tile([P, 2 * H2], F32, tag="uv2")
            nc.vector.tensor_copy(uv2_sb[:], uv2_ps[:])
            nc.vector.tensor_copy(pay_g2[:, NC1:NC1 + 2], uv2_sb[:, H2:])
            nc.vector.memset(pay_g2[:, NC1 + 2:], 0.0)
            dtap("d_xg", pay_g2[:])
            dtap("d_uv2", uv2_sb[:])
            ag_xg = allgather(pay_g2[:], P, NC1 + 8, "xg")
            xg_full = full_p.tile([P, C, NC1 + 8], BF16, tag="full520")
            nc.gpsimd.dma_start(xg_full[:], ag_xg[:].rearrange("(c p) f -> p c f", p=P))
            # wg2 stream (needed from here on)
            wg2_sb = w_str.tile([P, H2, 4, NC1], BF16, tag="wo1stream")
            for h in range(H2):
                nc.sync.dma_start(wg2_sb[:, h], wg2_view[h])

            # ======== GAT2: 2 heads; hcat2 AG split per head ========
            vbs2 = {h: col_extract_vb(xg_full[:, :, NC1 + h], f"g2_{h}")
                    for h in range(H2)}
            sm2 = {0: softmax_rows(uv2_sb[:, 0:1], vbs2.pop(0)[:], "g2")}
            ag_h2 = []
            uvo2h = []
            for h in range(H2):
                n_bf, rs = sm2.pop(h)
                if h == 0:
                    dtap("d_n20", n_bf[:])
                attT = att_p.tile([P, C, P], BF16, tag="attT")
                transpose8(n_bf[:], C, attT[:].rearrange("p c m -> p (c m)"))
                z_ps = ps_wh.tile([P, NC1], F32, tag="wh")
                for j in range(C):
                    nc.tensor.matmul(z_ps[:], attT[:, j, :],
                                     xg_full[:, j, :NC1],
                                     start=(j == 0), stop=(j == C - 1))
                z_bf = sc_bf.tile([P, NC1], BF16, tag="h0b")
                nc.vector.tensor_scalar(z_bf[:], z_ps[:], rs[:], None, op0=OP.mult)
                if h == 0:
                    dtap("d_z20", z_bf[:])
                zT = att_p.tile([P, 4, P], BF16, tag="xgT")
                transpose8(z_bf[:], 4, zT[:].rearrange("p c m -> p (c m)"))
                if h + 1 < H2:
                    sm2[h + 1] = softmax_rows(uv2_sb[:, h + 1:h + 2],
                                              vbs2.pop(h + 1)[:], "g2")
                o_ps = ps_wh.tile([P, NC1], F32, tag="wh")
                for j in range(4):
                    nc.tensor.matmul(o_ps[:], zT[:, j, :], wg2_sb[:, h, j, :],
                                     start=(j == 0), stop=(j == 3))
                pay = sc_bf.tile([P, NC1 + 8], BF16, tag="pay520",
                                 name=f"payh2_{h}")
                elu_store(o_ps[:], pay[:, :NC1], NC1)
                # u,v contributions of this half (Woa2 halves)
                junk2 = sc_bf.tile([P, NC1], BF16, tag="v1bf", name=f"jk2_{h}")
                uvh = sm.tile([P, 2], F32, tag="uvo2", name=f"uvo2_{h}")
                nc.vector.scalar_tensor_tensor(
                    junk2[:], pay[:, :NC1], 1.0,
                    woa2u_bc[:, h * NC1:(h + 1) * NC1], op0=OP.mult,
                    op1=OP.mult, accum_out=uvh[:, 0:1])
                nc.vector.scalar_tensor_tensor(
                    junk2[:], pay[:, :NC1], 1.0,
                    woa2v_bc[:, h * NC1:(h + 1) * NC1], op0=OP.mult,
                    op1=OP.mult, accum_out=uvh[:, 1:2])
                uvo2h.append(uvh)
                if h == 0:
                    nc.vector.memset(pay[:, NC1:], 0.0)
                else:
                    uvo2 = sm.tile([P, 2], F32, tag="uvo2f")
                    nc.vector.tensor_tensor(uvo2[:], uvo2h[0][:], uvo2h[1][:],
                                            OP.add)
                    nc.vector.tensor_copy(pay[:, NC1:NC1 + 1], uvo2[:, 1:2])
                    nc.vector.memset(pay[:, NC1 + 1:], 0.0)
                ag_h2.append(allgather(pay[:], P, NC1 + 8, f"h2_{h}"))

            # ======== GAT2 out-attention (o2) ========
            h2a_full = full_p.tile([P, C, NC1 + 8], BF16, tag="full520")
            nc.gpsimd.dma_start(h2a_full[:], ag_h2[0][:].rearrange("(c p) f -> p c f", p=P))
            h2b_full = full_p.tile([P, C, NC1 + 8], BF16, tag="full520")
            nc.gpsimd.dma_start(h2b_full[:], ag_h2[1][:].rearrange("(c p) f -> p c f", p=P))
            # Wo2 stream (during the AG)
            wo2_sb = w_str.tile([P, C, N], BF16, tag="wstream")
            nc.sync.dma_start(wo2_sb[:], wo2_view)
            vb = col_extract_vb(h2b_full[:, :, NC1], "o2")
            n_bf, rs = softmax_rows(uvo2[:, 0:1], vb[:], "o2")
            attT = att_p.tile([P, C, P], BF16, tag="attT")
            transpose8(n_bf[:], C, attT[:].rearrange("p c m -> p (c m)"))
            # z = att @ hcat2_full  (column halves from the two AGs)
            z_ps = ps_big.tile([P, N], F32, tag="big")
            for j in range(C):
                nc.tensor.matmul(z_ps[:, 0:512], attT[:, j, :],
                                 h2a_full[:, j, :NC1],
                                 start=(j == 0), stop=(j == C - 1))
            for j in range(C):
                nc.tensor.matmul(z_ps[:, 512:1024], attT[:, j, :],
                                 h2b_full[:, j, :NC1],
                                 start=(j == 0), stop=(j == C - 1))
            z_bf = sc_bf.tile([P, N], BF16, tag="zbf")
            nc.vector.tensor_scalar(z_bf[:], z_ps[:], rs[:], None, op0=OP.mult)
            zT = att_p.tile([P, C, P], BF16, tag="zT")
            transpose8(z_bf[:], C, zT[:].rearrange("p c m -> p (c m)"))
            o_ps = ps_big.tile([P, N], F32, tag="big")
            for j in range(C):
                for s in range(2):
                    nc.tensor.matmul(o_ps[:, s * 512:(s + 1) * 512],
                                     zT[:, j, :], wo2_sb[:, j, s * 512:(s + 1) * 512],
                                     start=(j == 0), stop=(j == C - 1))
            xg2_bf = sc_bf.tile([P, N], BF16, tag="obf")
            elu_store(o_ps[:], xg2_bf[:], N)
            dtap("d_xg2", xg2_bf[:])
            xg2T = att_p.tile([P, C, P], BF16, tag="zT")
            transpose8(xg2_bf[:], C, xg2T[:].rearrange("p c m -> p (c m)"))

            # ======== GCNII ========
            hi_ps = ps_wh.tile([P, HID], F32, tag="wh")
            for j in range(C):
                nc.tensor.matmul(hi_ps[:], xg2T[:, j, :], h0_full[:, j, :],
                                 start=(j == 0), stop=(j == C - 1))
            sf = sc_32.tile([P, HID], F32, tag="sf", bufs=1)
            nc.vector.scalar_tensor_tensor(sf[:], hi_ps[:], 9.0, h0f[:],
                                           op0=OP.mult, op1=OP.add)
            nc.vector.tensor_scalar(sf[:], sf[:], 0.1, None, op0=OP.mult)
            s_bf = sc_bf.tile([P, HID], BF16, tag="h0b")
            nc.vector.tensor_copy(s_bf[:], sf[:])
            dtap("d_sf", sf[:])
            ag_s = allgather(s_bf[:], P, HID, "s")
            s_full = full_p.tile([P, C, HID], BF16, tag="sfull", bufs=1)
            nc.gpsimd.dma_start(s_full[:], ag_s[:].rearrange("(c p) f -> p c f", p=P))
            fc1_sb = w_str.tile([P, 4, N], BF16, tag="wo1stream")
            nc.sync.dma_start(fc1_sb[:], fc1_view)
            mm_ps = ps_wh.tile([P, HID], F32, tag="wh")
            for c in range(C):
                nc.tensor.matmul(mm_ps[:], cw1T_sb[:, c, :], s_full[:, c, :],
                                 start=(c == 0), stop=(c == C - 1))
            hf = sc_32.tile([P, HID], F32, tag="s32")
            nc.vector.scalar_tensor_tensor(hf[:], sf[:], (1.0 - THETA2) / THETA2,
                                           mm_ps[:], op0=OP.mult, op1=OP.add)
            nc.vector.scalar_tensor_tensor(hf[:], hf[:], THETA2, h0f[:],
                                           op0=OP.mult, op1=OP.add)
            nc.vector.scalar_tensor_tensor(hf[:], hf[:], SLOPE, hf[:],
                                           op0=OP.mult, op1=OP.max)
            hb = sc_bf.tile([P, HID], BF16, tag="h0b")
            nc.vector.tensor_copy(hb[:], hf[:])
            hT = att_p.tile([P, 4, P], BF16, tag="xgT")
            transpose8(hb[:], 4, hT[:].rearrange("p c m -> p (c m)"))
            y_ps = ps_big.tile([P, N], F32, tag="big")
            for c in range(4):
                for s in range(2):
                    nc.tensor.matmul(y_ps[:, s * 512:(s + 1) * 512], hT[:, c, :],
                                     fc1_sb[:, c, s * 512:(s + 1) * 512],
                                     start=(c == 0), stop=(c == 3))
            y_sb = sc_32.tile([P, N], F32, tag="s32")
            nc.vector.scalar_tensor_tensor(y_sb[:], y_ps[:], 1.0, fc1b_bc[:],
                                           op0=OP.mult, op1=OP.add)
            nc.sync.dma_start(out_d.ap(), y_sb[:])


def _shard_inputs(inputs):
    f32 = lambda a: np.asarray(a, dtype=np.float32)
    bf = lambda a: np.ascontiguousarray(f32(a)).astype(ml_dtypes.bfloat16)
    x = f32(inputs["x"])
    adj = f32(inputs["adj"])
    x_bf = bf(x)
    xT_bf = np.ascontiguousarray(x_bf.T)
    cw1T = np.ascontiguousarray(bf(inputs["cw1"]).T)
    Wg1 = f32(inputs["Wg1"])
    ag1 = f32(inputs["ag1"])[:, :, 0]          # [5, 2048]
    Wa1 = np.stack([Wg1[h] @ ag1[h, :N] for h in range(H1)] +
                   [Wg1[h] @ ag1[h, N:] for h in range(H1)], axis=1)  # [1024, 10]
    Wg2 = f32(inputs["Wg2"])
    ag2 = f32(inputs["ag2"])[:, :, 0]          # [2, 1024]
    Wa2 = np.stack([Wg2[h] @ ag2[h, :NC1] for h in range(H2)] +
                   [Wg2[h] @ ag2[h, NC1:] for h in range(H2)], axis=1)  # [512, 4]
    Wo2 = f32(inputs["Wo2"])
    ao2 = f32(inputs["ao2"])[:, 0]             # [2048]
    Woa2 = np.stack([Wo2 @ ao2[:N], Wo2 @ ao2[N:]], axis=1)  # [1024, 2]
    shared = {
        "x_row": x_bf,
        "Wg1": bf(Wg1),
        "Wa1": bf(Wa1),
        "Wo1": bf(inputs["Wo1"]),
        "ao1": bf(f32(inputs["ao1"])[:, 0]),
        "Wg2": bf(Wg2),
        "Wa2": bf(Wa2),
        "Wo2": bf(Wo2),
        "Woa2": bf(Woa2),
        "fc0_w": bf(inputs["fc0_w"]),
        "fc0_b": bf(inputs["fc0_b"]),
        "fc1_w": bf(inputs["fc1_w"]),
        "fc1_b": bf(inputs["fc1_b"]),
    }
    in_maps = []
    for c in range(C):
        r0, r1 = c * P, (c + 1) * P
        m = dict(shared)
        m["xT_sl"] = np.ascontiguousarray(xT_bf[:, r0:r1])
        m["adj_r"] = np.ascontiguousarray(adj[r0:r1])
        m["cw1T_sl"] = np.ascontiguousarray(cw1T[:, r0:r1])
        in_maps.append(m)
    return in_maps


def kernel(**inputs) -> np.ndarray:
    if "nc" not in _CACHE:
        _CACHE["nc"] = _build()
    nc = _CACHE["nc"]
    in_maps = _shard_inputs(inputs)
    res = run_bass_kernel_spmd(nc, in_maps, core_ids=list(range(C)))
    out = np.concatenate([res.results[c]["out"] for c in range(C)], axis=0)
    return np.asarray(out, dtype=np.float32)
